# revision 1
# baseline (speedup 1.0000x reference)
"""CrissCrossAttention Trainium2 kernel.

Per-core: one batch b of x [C=512, HW=9216] (h-major pixels, p = h*96+w).

Math (reference):
  q = Wq x + bq ; k = Wk x + bk ; v = Wv x + bv        (1x1 convs)
  E_col[g,h] per w = sum_c k[c,g,w] q[c,h,w]  (diag g==h masked -inf)
  E_row[v,w] per h                                      (row logits)
  attn = softmax over concat(H' + W') per dest pixel
  out = gamma*(out_h + out_w) + x

The end-to-end time in this axon-tunneled setup is dominated by the
host<->device wire (~50-90MB/s shared), so the design minimizes bytes on
the wire and overlaps host work / upload / exec / download:

Host (single CPU core):
  - x' = x + gamma*bv (residual shift folding bv; bq/bk adjusted to match,
    v-path correction row -Wv(gamma*bv) added on device via K=1 matmul).
  - q/k projections computed on host (one batched sgemm) and shipped fp16
    [2*IC, HW] (19MB) instead of shipping fp32/fp16 x for them.
  - x' shipped as per-channel int8 [C, HW] + fp32 scales (19MB) for the
    v-path only; logits never see the int8 rounding.
  - residual add and int8 delta dequant happen on host in fp32.

Device (per core, Tile framework):
  - q/k fp16 on the wire, upconverted to fp32 for PE logit matmuls (PE runs
    only proven dtypes: fp32 logits/masks, bf16 value path; fp16 PE operands
    showed sporadic NRT exec-unit faults).
  - v = Wv x' via bf16 matmuls from dequantized int8 x'.
  - P = exp(logits) unnormalized bf16 (values up to e^40 overflow fp16);
    denominators D = colsum + rowsum via ones-matmuls; Rg = gamma/D.
  - U_colT(w) / U_rowT(h) -> [96, C] bf16 scratch in DRAM; final pass
    DMA-transposes them back, sums to delta = gamma*(out_h+out_w), and emits
    delta as per-channel int8 [C, HW] + fp32 amax (19MB download).

Dispatch: replicates run_bass_kernel_spmd's axon path (shard_map +
_bass_exec_p custom call) but builds the jitted executables ONCE and caches
them; donated zero output buffers are created on-device (no host upload);
the 8 cores run as 2 groups of 4 so the second group's upload/exec overlaps
the first group's download; all shard fetches share one thread pool so the
wire stays saturated. On a wedged NeuronCore the backend is torn down,
rebuilt, and the call retried before falling back to run_bass_kernel_spmd.
"""

import numpy as np
import ml_dtypes
from concurrent.futures import ThreadPoolExecutor

C, IC, H, W = 512, 64, 96, 96
HW = H * W  # 9216
NB = 18  # 512-wide pixel blocks
NCORES = 8
BF = ml_dtypes.bfloat16


def _build(gamma_f: float):
    from contextlib import ExitStack
    import concourse.bass as bass
    import concourse.bacc as bacc
    import concourse.tile as tile
    from concourse import mybir

    f32 = mybir.dt.float32
    f16 = mybir.dt.float16
    bf16 = mybir.dt.bfloat16
    AF = mybir.ActivationFunctionType

    nc = bacc.Bacc("TRN2", target_bir_lowering=False, debug=False)

    i8 = mybir.dt.int8
    xq_d = nc.dram_tensor("xq", [C, HW], i8, kind="ExternalInput").ap()
    xs_d = nc.dram_tensor("xs", [128, 4], f32, kind="ExternalInput").ap()
    qk_d = nc.dram_tensor("qk", [2 * IC, HW], f16, kind="ExternalInput").ap()
    wv_d = nc.dram_tensor("wvT", [4, 128, C], bf16, kind="ExternalInput").ap()
    mwvd_d = nc.dram_tensor("mwvd", [1, C], bf16, kind="ExternalInput").ap()
    ib_d = nc.dram_tensor("ib", [96, 96], f32, kind="ExternalInput").ap()
    negib_d = nc.dram_tensor("negib", [96, 96], f32, kind="ExternalInput").ap()
    outq_d = nc.dram_tensor("outq", [C, HW], i8, kind="ExternalOutput").ap()
    outs_d = nc.dram_tensor("outs", [C, 1], f32, kind="ExternalOutput").ap()

    vt_d = nc.dram_tensor("vt_scratch", [HW, C], bf16, kind="Internal").ap()
    uc_d = nc.dram_tensor("uc_scratch", [HW, C], bf16, kind="Internal").ap()
    ur_d = nc.dram_tensor("ur_scratch", [HW, C], bf16, kind="Internal").ap()
    sc_d = nc.dram_tensor("sc_scratch", [1, HW], f32, kind="Internal").ap()
    sr_d = nc.dram_tensor("sr_scratch", [1, HW], f32, kind="Internal").ap()

    with tile.TileContext(nc) as tc, ExitStack() as top:
        const = top.enter_context(tc.tile_pool(name="const", bufs=1))
        persist = top.enter_context(tc.tile_pool(name="persist", bufs=1))

        wv_sb = const.tile([128, 4, C], bf16)
        nc.sync.dma_start(out=wv_sb, in_=wv_d.rearrange("c p m -> p c m"))
        mwvd_sb = const.tile([1, C], bf16)
        nc.sync.dma_start(out=mwvd_sb, in_=mwvd_d)
        ib_sb = const.tile([96, 96], f32)
        nc.sync.dma_start(out=ib_sb, in_=ib_d)
        negib_sb = const.tile([96, 96], f32)
        nc.sync.dma_start(out=negib_sb, in_=negib_d)
        xs_sb = const.tile([128, 4], f32)
        nc.sync.dma_start(out=xs_sb, in_=xs_d)
        ones1_sb = const.tile([1, 128], bf16)
        nc.vector.memset(ones1_sb, 1.0)
        ones96_sb = const.tile([96, 1], bf16)
        nc.vector.memset(ones96_sb, 1.0)

        # host-computed q/k projections: fp16 on the wire, fp32 on the PE
        # (fp32 logit matmuls match the proven baseline PE usage; fp16
        # operands on the TensorEngine showed sporadic exec-unit faults)
        q_sb = persist.tile([IC, HW], f32)
        k_sb = persist.tile([IC, HW], f32)
        with tc.tile_pool(name="qkstage", bufs=1) as qkst:
            qh_sb = qkst.tile([IC, HW], f16)
            nc.sync.dma_start(out=qh_sb, in_=qk_d[0:IC, :])
            nc.scalar.copy(q_sb, qh_sb)
            kh_sb = qkst.tile([IC, HW], f16)
            nc.sync.dma_start(out=kh_sb, in_=qk_d[IC:2 * IC, :])
            nc.vector.tensor_copy(k_sb, kh_sb)
        pc_sb = persist.tile([96, HW], bf16)  # exp(col logits), [g, (w,h)] w-major
        pr_sb = persist.tile([96, HW], bf16)  # exp(row logits), [v, (h,w)] h-major
        rg_sb = persist.tile([96, 96], f32)  # gamma/D, [h, w]
        rgt_sb = persist.tile([96, 96], f32)  # [w, h]

        # ---------------- Phase P: v projection (int8 x dequant) ----------------
        xv = xq_d.rearrange("(cc p) n -> p cc n", p=128)
        vtw = vt_d.rearrange("(q pt p) c -> q p pt c", pt=4, p=128)
        with ExitStack() as ph, tc.tile_pool(name="pstage", bufs=2) as stage, \
                tc.tile_pool(name="ppsum", bufs=2, space="PSUM") as psv:
            for nb in range(NB):
                s, e = nb * 512, (nb + 1) * 512
                xq = stage.tile([128, 4, 512], i8, tag="xq")
                nc.sync.dma_start(out=xq, in_=xv[:, :, s:e])
                xbb = stage.tile([128, 4, 512], bf16, tag="xbb")
                for cc in range(4):
                    if (nb + cc) % 2 == 0:
                        nc.vector.tensor_scalar_mul(xbb[:, cc, :], xq[:, cc, :],
                                                    xs_sb[:, cc:cc + 1])
                    else:
                        nc.scalar.activation(xbb[:, cc, :], xq[:, cc, :],
                                             AF.Copy, scale=xs_sb[:, cc:cc + 1])
                vstage = stage.tile([128, 4, 512], bf16, tag="vst")
                for pt in range(4):
                    pv = psv.tile([128, 512], f32, tag="pv")
                    for cc in range(4):
                        nc.tensor.matmul(pv, lhsT=xbb[:, cc, pt * 128:(pt + 1) * 128],
                                         rhs=wv_sb[:, cc, :], start=(cc == 0), stop=False)
                    nc.tensor.matmul(pv, lhsT=ones1_sb, rhs=mwvd_sb, start=False, stop=True)
                    if pt % 2 == 0:
                        nc.scalar.copy(vstage[:, pt, :], pv)
                    else:
                        nc.vector.tensor_copy(vstage[:, pt, :], pv)
                nc.sync.dma_start(out=vtw[nb], in_=vstage)

        # ---------------- Phase L: logits, exp, sums ----------------
        kc = k_sb.rearrange("c (g w) -> c g w", w=96)
        qc = q_sb.rearrange("c (g w) -> c g w", w=96)
        with ExitStack() as ph, tc.tile_pool(name="lpsum", bufs=4, space="PSUM") as pse, \
                tc.tile_pool(name="spsum", bufs=2, space="PSUM") as pss, \
                tc.tile_pool(name="sstage", bufs=2) as sst:
            for hg in range(24):
                pe4 = pse.tile([96, 384], f32, tag="pe")
                for hi in range(4):
                    h = hg * 4 + hi
                    sl = slice(hi * 96, (hi + 1) * 96)
                    nc.tensor.matmul(pe4[:, sl], lhsT=k_sb[:, h * 96:(h + 1) * 96],
                                     rhs=q_sb[:, h * 96:(h + 1) * 96],
                                     start=True, stop=True)
                nc.scalar.activation(pr_sb[:, hg * 384:(hg + 1) * 384], pe4, AF.Exp)
            for wg in range(24):
                pe4 = pse.tile([96, 384], f32, tag="pe")
                for wi in range(4):
                    w = wg * 4 + wi
                    sl = slice(wi * 96, (wi + 1) * 96)
                    nc.tensor.matmul(pe4[:, sl], lhsT=kc[:, :, w], rhs=qc[:, :, w],
                                     start=True, stop=False)
                    nc.tensor.matmul(pe4[:, sl], lhsT=ib_sb, rhs=negib_sb,
                                     start=False, stop=True)
                nc.scalar.activation(pc_sb[:, wg * 384:(wg + 1) * 384], pe4, AF.Exp)
            for j in range(NB):
                s, e = j * 512, (j + 1) * 512
                p1 = pss.tile([1, 512], f32, tag="p1")
                nc.tensor.matmul(p1, lhsT=ones96_sb, rhs=pc_sb[:, s:e], start=True, stop=True)
                t1 = sst.tile([1, 512], f32, tag="t1")
                nc.vector.tensor_copy(t1, p1)
                nc.sync.dma_start(out=sc_d[:, s:e], in_=t1)
                p2 = pss.tile([1, 512], f32, tag="p2")
                nc.tensor.matmul(p2, lhsT=ones96_sb, rhs=pr_sb[:, s:e], start=True, stop=True)
                t2 = sst.tile([1, 512], f32, tag="t2")
                nc.scalar.copy(t2, p2)
                nc.sync.dma_start(out=sr_d[:, s:e], in_=t2)

        # ---------------- Phase D: denominators -> Rg, RgT ----------------
        with ExitStack() as ph, tc.tile_pool(name="dsmall", bufs=1) as dsm, \
                tc.tile_pool(name="dpsum", bufs=1, space="PSUM") as dps:
            sct = dsm.tile([96, 96], f32)  # [w, h]
            nc.sync.dma_start(out=sct, in_=sc_d.rearrange("one (w h) -> (one w) h", h=96))
            srt = dsm.tile([96, 96], f32)  # [h, w]
            nc.sync.dma_start(out=srt, in_=sr_d.rearrange("one (h w) -> (one h) w", w=96))
            ptr = dps.tile([96, 96], f32)
            nc.tensor.transpose(ptr, sct, ib_sb)  # -> [h, w]
            d_sb = dsm.tile([96, 96], f32)
            nc.vector.tensor_add(d_sb, ptr, srt)
            r_sb = dsm.tile([96, 96], f32)
            nc.vector.reciprocal(r_sb, d_sb)
            nc.scalar.activation(rg_sb, r_sb, AF.Copy, scale=float(gamma_f))
            ptr2 = dps.tile([96, 96], f32)
            nc.tensor.transpose(ptr2, rg_sb, ib_sb)
            nc.vector.tensor_copy(rgt_sb, ptr2)

        # ------- Phases C+R interleaved: column + row attention -------
        vtc = vt_d.rearrange("(g wg wi) c -> wg g wi c", wg=24, wi=4)
        ucw = uc_d.rearrange("(h wg wi) c -> wg h wi c", wg=24, wi=4)
        vtr = vt_d.rearrange("(hg hi v) c -> hg v hi c", hg=24, hi=4)
        urw = ur_d.rearrange("(hg hi w) c -> hg w hi c", hg=24, hi=4)
        with ExitStack() as ph, tc.tile_pool(name="crstage", bufs=4) as cst, \
                tc.tile_pool(name="cpsum", bufs=3, space="PSUM") as psu, \
                tc.tile_pool(name="rpsum", bufs=3, space="PSUM") as psr:
            for grp in range(24):
                wg = grp
                vc = cst.tile([96, 4, C], bf16, tag="vc")
                nc.sync.dma_start(out=vc, in_=vtc[wg])
                uc = cst.tile([96, 4, C], bf16, tag="uc")
                for wi in range(4):
                    w = wg * 4 + wi
                    pu = psu.tile([96, C], f32, tag="pu")
                    nc.tensor.matmul(pu, lhsT=pc_sb[:, w * 96:(w + 1) * 96],
                                     rhs=vc[:, wi, :], start=True, stop=True)
                    if w % 2 == 0:
                        nc.scalar.activation(uc[:, wi, :], pu, AF.Copy,
                                             scale=rg_sb[:, w:w + 1])
                    else:
                        nc.vector.tensor_scalar_mul(uc[:, wi, :], pu, rg_sb[:, w:w + 1])
                nc.sync.dma_start(out=ucw[wg], in_=uc)
                hg = grp
                vr = cst.tile([96, 4, C], bf16, tag="vr")
                nc.sync.dma_start(out=vr, in_=vtr[hg])
                ur = cst.tile([96, 4, C], bf16, tag="ur")
                for hi in range(4):
                    h = hg * 4 + hi
                    pu = psr.tile([96, C], f32, tag="pur")
                    nc.tensor.matmul(pu, lhsT=pr_sb[:, h * 96:(h + 1) * 96],
                                     rhs=vr[:, hi, :], start=True, stop=True)
                    if h % 2 == 0:
                        nc.scalar.activation(ur[:, hi, :], pu, AF.Copy,
                                             scale=rgt_sb[:, h:h + 1])
                    else:
                        nc.vector.tensor_scalar_mul(ur[:, hi, :], pu, rgt_sb[:, h:h + 1])
                nc.sync.dma_start(out=urw[hg], in_=ur)

        # ------- Phase F: delta = uc+ur, per-channel int8 quantization -------
        # delta already carries the gamma/D scaling; residual add moves to host.
        # q = round-ish(delta * 126/amax_c), host dequant s_c = amax_c/126.
        with ExitStack() as ph, tc.tile_pool(name="fstage", bufs=3) as fst, \
                tc.tile_pool(name="fsball", bufs=2) as fsb:
            for cc in range(4):
                cs = slice(cc * 128, (cc + 1) * 128)
                sball = fsb.tile([128, HW], bf16, tag="sball")
                for hb in range(6):
                    r0 = hb * 1536
                    uct = fst.tile([128, 1536], bf16, tag="uct")
                    nc.sync.dma_start(out=uct, in_=uc_d[r0:r0 + 1536, cs], transpose=True)
                    urt = fst.tile([128, 1536], bf16, tag="urt")
                    nc.sync.dma_start(out=urt, in_=ur_d[r0:r0 + 1536, cs], transpose=True)
                    if (cc + hb) % 2 == 0:
                        nc.gpsimd.tensor_add(sball[:, r0:r0 + 1536], uct, urt)
                    else:
                        nc.vector.tensor_add(sball[:, r0:r0 + 1536], uct, urt)
                amax = fst.tile([128, 1], f32, tag="amax")
                nc.vector.tensor_reduce(amax, sball,
                                        axis=mybir.AxisListType.X,
                                        op=mybir.AluOpType.max,
                                        apply_absolute_value=True)
                nc.sync.dma_start(out=outs_d[cs, :], in_=amax)
                rinv = fst.tile([128, 1], f32, tag="rinv")
                nc.vector.reciprocal(rinv, amax)
                rs = fst.tile([128, 1], f32, tag="rs")
                # rs = 126/amax  (margin below 127 so reciprocal error can't
                # push the max element past int8 range)
                nc.scalar.activation(rs, rinv, AF.Copy, scale=126.0)
                for hb in range(6):
                    r0 = hb * 1536
                    q8 = fst.tile([128, 1536], i8, tag="q8")
                    if hb % 2 == 0:
                        nc.vector.tensor_scalar_mul(q8, sball[:, r0:r0 + 1536], rs)
                    else:
                        nc.scalar.activation(q8, sball[:, r0:r0 + 1536],
                                             AF.Copy, scale=rs)
                    nc.sync.dma_start(out=outq_d[cs, r0:r0 + 1536], in_=q8)

    nc.compile()
    return nc


NGROUPS = int(__import__("os").environ.get("KERNEL_NGROUPS", "2"))


def _make_runner(gamma_f: float, ngroups: int = NGROUPS):
    """Build the Bass module once and wrap it in cached jitted dispatchers
    (the axon run_bass_kernel_spmd path, minus the per-call retrace, minus
    the host-side zero-output upload). The 8 cores are split into `ngroups`
    independent dispatch groups so a later group's upload/exec overlaps an
    earlier group's download through the shared tunnel."""
    import jax
    import jax.numpy as jnp
    from jax.sharding import Mesh, PartitionSpec, NamedSharding
    try:
        from jax.experimental.shard_map import shard_map
    except ImportError:
        from jax.shard_map import shard_map
    from concourse import bass2jax, mybir
    from concourse.bass2jax import _bass_exec_p, install_neuronx_cc_hook

    nc = _build(gamma_f)
    install_neuronx_cc_hook()
    if nc.dbg_addr is not None and nc.dbg_callbacks:
        raise RuntimeError("dbg callbacks unsupported in cached dispatch")

    partition_name = nc.partition_id_tensor.name if nc.partition_id_tensor else None
    in_names, out_names, out_avals = [], [], []
    for alloc in nc.m.functions[0].allocations:
        if not isinstance(alloc, mybir.MemoryLocationSet):
            continue
        name = alloc.memorylocations[0].name
        if alloc.kind == "ExternalInput":
            if name != partition_name:
                in_names.append(name)
        elif alloc.kind == "ExternalOutput":
            out_names.append(name)
            out_avals.append(jax.core.ShapedArray(
                tuple(alloc.tensor_shape), mybir.dt.np(alloc.dtype)))
    n_params = len(in_names)
    n_outs = len(out_names)
    bind_in_names = tuple(in_names + out_names
                          + ([partition_name] if partition_name else []))

    def _body(*args):
        operands = list(args)
        if partition_name is not None:
            operands.append(bass2jax.partition_id_tensor())
        outs = _bass_exec_p.bind(
            *operands,
            out_avals=tuple(out_avals),
            in_names=bind_in_names,
            out_names=tuple(out_names),
            lowering_input_output_aliases=(),
            sim_require_finite=True,
            sim_require_nnan=True,
            nc=nc,
        )
        return tuple(outs)

    devices = jax.devices()[:NCORES]
    assert len(devices) == NCORES, f"need {NCORES} devices, have {len(jax.devices())}"
    assert NCORES % ngroups == 0
    gsize = NCORES // ngroups
    in_specs = (PartitionSpec("core"),) * (n_params + n_outs)
    out_specs = (PartitionSpec("core"),) * n_outs
    donate = tuple(range(n_params, n_params + n_outs))
    groups = []
    for gi in range(ngroups):
        mesh = Mesh(np.asarray(devices[gi * gsize:(gi + 1) * gsize]), ("core",))
        nshard = NamedSharding(mesh, PartitionSpec("core"))
        fn = jax.jit(
            shard_map(_body, mesh=mesh, in_specs=in_specs, out_specs=out_specs,
                      check_rep=False),
            donate_argnums=donate,
            keep_unused=True,
        )
        # Donated zero output buffers, created on-device (no host upload).
        zero_fns = [
            jax.jit(
                (lambda shape, dt: (lambda: jnp.zeros(shape, dt)))(
                    (gsize * av.shape[0],) + tuple(av.shape[1:]), av.dtype),
                out_shardings=nshard)
            for av in out_avals
        ]
        groups.append(dict(fn=fn, zero_fns=zero_fns, nshard=nshard))
    return dict(nc=nc, groups=groups, gsize=gsize,
                in_names=in_names, out_names=out_names)


_cache: dict = {}
_bufs: dict = {}

try:
    import numba

    @numba.njit(cache=True, fastmath=True, nogil=True)
    def _nb_quant(xb, q8, srow):
        # q8 = rint(xb * 127/amax_row); srow = amax_row/127 (per channel)
        for c in range(xb.shape[0]):
            row = xb[c]
            m = 1e-20
            for p in range(row.shape[0]):
                a = abs(row[p])
                if a > m:
                    m = a
            s = 127.0 / m
            qrow = q8[c]
            for p in range(row.shape[0]):
                qrow[p] = np.int8(np.rint(row[p] * s))
            srow[c] = m * (1.0 / 127.0)

    @numba.njit(cache=True, fastmath=True, nogil=True)
    def _nb_dequant(acc, q8, srow):
        # acc += q8 * srow[c]  (per-channel scale)
        for c in range(acc.shape[0]):
            s = srow[c]
            arow = acc[c]
            qrow = q8[c]
            for p in range(arow.shape[0]):
                arow[p] += qrow[p] * s
except Exception:  # numba unavailable: numpy fallbacks below
    _nb_quant = None
    _nb_dequant = None


def _prep_shared(Wq, bq, Wk, bk, Wv, bv, delta):
    Wq = np.asarray(Wq, np.float32)
    Wk = np.asarray(Wk, np.float32)
    Wv = np.asarray(Wv, np.float32)
    dev = dict(
        wvT=np.ascontiguousarray(Wv.T).astype(BF).reshape(4, 128, C),
        mwvd=(-(Wv @ delta)).astype(BF).reshape(1, C),
        ib=np.eye(96, dtype=np.float32),
        negib=np.eye(96, dtype=np.float32) * -1e30,
    )
    Wqk = np.vstack([Wq, Wk])  # (2*IC, C)
    bqk = np.concatenate([np.asarray(bq, np.float32) - Wq @ delta,
                          np.asarray(bk, np.float32) - Wk @ delta])
    return dev, Wqk, bqk


def _getbuf(name, shape, dtype):
    b = _bufs.get(name)
    if b is None or b.shape != shape or b.dtype != dtype:
        b = _bufs[name] = np.empty(shape, dtype)
    return b


_PROF = bool(__import__("os").environ.get("KERNEL_PROF"))


def _run_fast(runner, x32, delta, shared, Wqk, bqk, B):
    import jax
    import time as _t
    _t0 = _t.time()
    _mark = (lambda m: print(f"  [{_t.time()-_t0:7.3f}] {m}", flush=True)) \
        if _PROF else (lambda m: None)
    groups = runner["groups"]
    gsize = runner["gsize"]
    xd32 = np.empty((B, C, HW), np.float32)  # x + gamma*bv: residual base
    xq8 = _getbuf("xq8", (B, C, HW), np.int8)
    qk16 = _getbuf("qk16", (B, 2 * IC, HW), np.float16)
    xs_np = _getbuf("xs", (B, 128, 4), np.float32)
    wc = runner.get("wcache")
    fresh_w = not (wc is not None
                   and all(np.array_equal(shared[n], wc[0][n]) for n in shared))
    if fresh_w:
        dev_w_g = []
    qi = runner["out_names"].index("outq")
    si = runner["out_names"].index("outs")

    scr = _getbuf("scr", (C, HW), np.float32)
    devices = jax.devices()[:NCORES]

    srow = _getbuf("srow", (B, C), np.float32)

    def quant_b(b):
        xb = xd32[b]
        if _nb_quant is not None:
            _nb_quant(xb, xq8[b], srow[b])
        else:
            amax = np.maximum(xb.max(axis=1), -xb.min(axis=1))
            np.maximum(amax, 1e-20, out=amax)
            srow[b] = amax * (1.0 / 127.0)
            np.multiply(xb, (127.0 / amax)[:, None], out=scr)
            np.rint(scr, out=scr)
            xq8[b] = scr
        xs_np[b] = srow[b].reshape(4, 128).T

    # single CPU core: keep prep serial (threads only help I/O waits below)
    for b in range(B):
        np.add(x32[b], delta[:, None], out=xd32[b])
    _mark("add done")
    disp = []
    with ThreadPoolExecutor(NCORES) as ex:
        for gi, gr in enumerate(groups):
            b0, b1 = gi * gsize, (gi + 1) * gsize
            # stream each batch's int8 plane up as soon as it's quantized
            xparts = []
            for b in range(b0, b1):
                quant_b(b)
                xparts.append(jax.device_put(xq8[b], devices[b]))
            xg = jax.make_array_from_single_device_arrays(
                (gsize * C, HW), gr["nshard"], xparts)
            _mark(f"g{gi} xq staged")
            # this group's q/k projection; overlaps the int8 upload, and
            # streams each batch's plane up as soon as it's computed
            qparts = []
            for b in range(b0, b1):
                np.add(np.matmul(Wqk, xd32[b]), bqk[:, None], out=qk16[b],
                       casting="unsafe")
                qparts.append(jax.device_put(qk16[b], devices[b]))
            qg = jax.make_array_from_single_device_arrays(
                (gsize * 2 * IC, HW), gr["nshard"], qparts)
            _mark(f"g{gi} qk staged")
            sg = jax.device_put(xs_np[b0:b1].reshape(gsize * 128, 4),
                                gr["nshard"])
            per_call = {"xq": xg, "qk": qg, "xs": sg}
            if fresh_w:
                dev_w = {n: jax.device_put(np.concatenate([w] * gsize, axis=0),
                                           gr["nshard"])
                         for n, w in shared.items()}
                dev_w_g.append(dev_w)
            else:
                dev_w = wc[1][gi]
            args = [per_call.get(name) if name in per_call else dev_w[name]
                    for name in runner["in_names"]]
            zeros = gr.pop("next_zeros", None) or [zf() for zf in gr["zero_fns"]]
            disp.append(gr["fn"](*args, *zeros))
            _mark(f"g{gi} dispatched")
        if fresh_w:
            runner["wcache"] = ({n: np.copy(w) for n, w in shared.items()},
                                dev_w_g)
        for gi, gr in enumerate(groups):
            # prefetch next call's donated zero buffers; overlaps exec/fetch
            gr["next_zeros"] = [zf() for zf in gr["zero_fns"]]

        jobs = []
        for gi, out_arrs in enumerate(disp):
            jobs.extend((gi, s) for s in out_arrs[qi].addressable_shards)

        def fetch(arg):
            gi, s = arg
            q = np.asarray(s.data)  # (C, HW) int8; blocks until device done
            # scales for the group: first caller pays the (tiny) fetch, jax
            # caches the host copy for the rest
            snp = np.asarray(disp[gi][si]).reshape(gsize, C)
            b = gi * gsize + (s.index[0].start or 0) // C
            sc = snp[(s.index[0].start or 0) // C] * (1.0 / 126.0)
            if _nb_dequant is not None:
                _nb_dequant(xd32[b], q, sc)
            else:
                xd32[b] += q * sc[:, None]

        list(ex.map(fetch, jobs))
        _mark("all fetched")
    return xd32


def _run_fallback(nc, x32, delta, shared, Wqk, bqk, B):
    from concourse.bass_utils import run_bass_kernel_spmd
    xd32 = x32 + delta[None, :, None]
    in_maps = []
    for b in range(B):
        xb = xd32[b]
        amax = np.maximum(np.abs(xb).max(axis=1), 1e-20)
        in_maps.append(dict(
            shared,
            xq=np.rint(xb * (127.0 / amax)[:, None]).astype(np.int8),
            xs=np.ascontiguousarray((amax / 127.0).reshape(4, 128).T),
            qk=(Wqk @ xb + bqk[:, None]).astype(np.float16),
        ))
    res = run_bass_kernel_spmd(nc, in_maps, core_ids=list(range(B)))
    for b in range(B):
        s = res.results[b]["outs"].reshape(C, 1) * (1.0 / 126.0)
        xd32[b] += res.results[b]["outq"] * s
    return xd32


def kernel(x, Wq, bq, Wk, bk, Wv, bv, gamma):
    x = np.asarray(x)
    B = x.shape[0]
    assert B == NCORES, f"expected B={NCORES}, got {B}"
    g = float(np.asarray(gamma).reshape(-1)[0])
    delta = (g * np.asarray(bv, np.float64)).astype(np.float32)
    x32 = np.asarray(x, np.float32).reshape(B, C, HW)
    shared, Wqk, bqk = _prep_shared(Wq, bq, Wk, bk, Wv, bv, delta)

    key = round(g, 9)
    if key not in _cache:
        _cache[key] = _make_runner(g)
    runner = _cache[key]

    globals()["_last_exec_ns"] = None
    globals()["_last_trace"] = None
    try:
        res = _run_fast(runner, x32, delta, shared, Wqk, bqk, B)
    except Exception:
        import os, sys, time, traceback
        traceback.print_exc()
        if os.environ.get("KERNEL_NO_FALLBACK"):
            raise
        # A wedged NeuronCore (NRT_EXEC_UNIT_UNRECOVERABLE) persists for the
        # life of the PJRT client: tear the backend down, rebuild the runner
        # (terminal-side reconnect resets the cores), and retry once.
        print("kernel: fast dispatch failed; resetting backend", file=sys.stderr)
        try:
            import jax._src.xla_bridge as _xb
            _xb._clear_backends()
            import jax
            jax.clear_caches()
            time.sleep(2.0)
            _cache.clear()
            _cache[key] = runner = _make_runner(g)
            res = _run_fast(runner, x32, delta, shared, Wqk, bqk, B)
        except Exception:
            traceback.print_exc()
            print("kernel: retry failed; final fallback", file=sys.stderr)
            res = _run_fallback(runner["nc"], x32, delta, shared, Wqk, bqk, B)
    return res.reshape(B, C, H, W)



# revision 4
# speedup vs baseline: 1.4827x; 1.4827x over previous
"""CrissCrossAttention Trainium2 kernel.

Per-core: one batch b of x [C=512, HW=9216] (h-major pixels, p = h*96+w).

Math (reference):
  q = Wq x + bq ; k = Wk x + bk ; v = Wv x + bv        (1x1 convs)
  E_col[g,h] per w = sum_c k[c,g,w] q[c,h,w]  (diag g==h masked -inf)
  E_row[v,w] per h                                      (row logits)
  attn = softmax over concat(H' + W') per dest pixel
  out = gamma*(out_h + out_w) + x

The end-to-end time in this axon-tunneled setup is dominated by the
host<->device wire (~45-90MB/s shared, up+down combined), so the design
minimizes bytes on the wire and overlaps host work / upload / exec /
download.  Wire budget per batch: 5.9MB up + 3.6MB down (vs 11.8MB in the
naive int8-x + fp16-qk + int8-out layout).

Host (single CPU core):
  - x' = x + gamma*bv (residual shift folding bv; v-path correction row
    -Wv(gamma*bv) added on device via K=1 matmul; bq/bk adjusted inside the
    correction stream).
  - x' shipped as per-channel int8 [C, HW] + fp32 scales.
  - q/k are NOT shipped.  The device computes qk = Wqk @ xhat in fp32 (PE),
    where xhat is the int8-dequantized x'.  The host ships a small int8
    correction stream cq = quant(Wqk @ (x' - xhat) + bqk) [2*IC, HW]
    (1.2MB).  Host and device both evaluate Wqk @ xhat in fp32, so adding
    the correction reconstructs q/k to ~fp32 accuracy at 1/2 the bytes of
    fp16 q/k shipping.  Host pays one [128,512]x[512,9216] sgemm on the
    quantization residual (same flops as shipping q/k directly would).
  - residual add and the output-delta dequant happen on host in fp32.

Device (per core, Tile framework):
  - Phase P: dequant int8 x' -> fp32 x32 and bf16 xbb; q/k = fp32 PE
    matmuls of wqkT x32 + int8 correction; v = Wv xbb via bf16 matmuls.
  - P = exp(logits) unnormalized bf16 (values up to e^40 overflow fp16);
    denominators D = colsum + rowsum via ones-matmuls; Rg = gamma/D.
  - U_colT(w) / U_rowT(h) -> [96, C] bf16 scratch in DRAM; final pass
    DMA-transposes them back, sums to delta = gamma*(out_h+out_w), and
    emits delta as int6 (4 values packed into 3 bytes) with per-channel
    per-512px-block fp32 amax [C, 18].

Dispatch: replicates run_bass_kernel_spmd's axon path (shard_map +
_bass_exec_p custom call) but builds the jitted executables ONCE and caches
them; donated zero output buffers are created on-device (no host upload);
the 8 cores run as 2 groups of 4 so the second group's upload/exec overlaps
the first group's download; all shard fetches share one thread pool so the
wire stays saturated. On a wedged NeuronCore the backend is torn down,
rebuilt, and the call retried before falling back to run_bass_kernel_spmd.
"""

import numpy as np
import ml_dtypes
from concurrent.futures import ThreadPoolExecutor

C, IC, H, W = 512, 64, 96, 96
HW = H * W  # 9216
NB = 18  # 512-wide pixel blocks
NCORES = 8
BF = ml_dtypes.bfloat16
QMARGIN = 30.5  # int6 quant margin (|q| <= 31)
PACKW = HW // 4 * 3  # 6912 packed bytes per channel row


def _build(gamma_f: float):
    from contextlib import ExitStack
    import concourse.bass as bass
    import concourse.bacc as bacc
    import concourse.tile as tile
    from concourse import mybir

    f32 = mybir.dt.float32
    bf16 = mybir.dt.bfloat16
    i8 = mybir.dt.int8
    u8 = mybir.dt.uint8
    AF = mybir.ActivationFunctionType
    OP = mybir.AluOpType

    nc = bacc.Bacc("TRN2", target_bir_lowering=False, debug=False)

    xq_d = nc.dram_tensor("xq", [C, HW], i8, kind="ExternalInput").ap()
    xs_d = nc.dram_tensor("xs", [128, 4], f32, kind="ExternalInput").ap()
    cq_d = nc.dram_tensor("cq", [2 * IC, HW], i8, kind="ExternalInput").ap()
    cs_d = nc.dram_tensor("cs", [IC, 2], f32, kind="ExternalInput").ap()
    wqkT_d = nc.dram_tensor("wqkT", [4, 128, 2 * IC], f32, kind="ExternalInput").ap()
    wv_d = nc.dram_tensor("wvT", [4, 128, C], bf16, kind="ExternalInput").ap()
    mwvd_d = nc.dram_tensor("mwvd", [1, C], bf16, kind="ExternalInput").ap()
    ib_d = nc.dram_tensor("ib", [96, 96], f32, kind="ExternalInput").ap()
    negib_d = nc.dram_tensor("negib", [96, 96], f32, kind="ExternalInput").ap()
    outp_d = nc.dram_tensor("outp", [C, PACKW], u8, kind="ExternalOutput").ap()
    outs_d = nc.dram_tensor("outs", [C, NB], f32, kind="ExternalOutput").ap()

    vt_d = nc.dram_tensor("vt_scratch", [HW, C], bf16, kind="Internal").ap()
    uc_d = nc.dram_tensor("uc_scratch", [HW, C], bf16, kind="Internal").ap()
    ur_d = nc.dram_tensor("ur_scratch", [HW, C], bf16, kind="Internal").ap()
    sc_d = nc.dram_tensor("sc_scratch", [1, HW], f32, kind="Internal").ap()
    sr_d = nc.dram_tensor("sr_scratch", [1, HW], f32, kind="Internal").ap()

    with tile.TileContext(nc) as tc, ExitStack() as top:
        const = top.enter_context(tc.tile_pool(name="const", bufs=1))
        persist = top.enter_context(tc.tile_pool(name="persist", bufs=1))

        wv_sb = const.tile([128, 4, C], bf16)
        nc.sync.dma_start(out=wv_sb, in_=wv_d.rearrange("c p m -> p c m"))
        wqkT_sb = const.tile([128, 4, 2 * IC], f32)
        nc.sync.dma_start(out=wqkT_sb, in_=wqkT_d.rearrange("c p m -> p c m"))
        mwvd_sb = const.tile([1, C], bf16)
        nc.sync.dma_start(out=mwvd_sb, in_=mwvd_d)
        ib_sb = const.tile([96, 96], f32)
        nc.sync.dma_start(out=ib_sb, in_=ib_d)
        negib_sb = const.tile([96, 96], f32)
        nc.sync.dma_start(out=negib_sb, in_=negib_d)
        xs_sb = const.tile([128, 4], f32)
        nc.sync.dma_start(out=xs_sb, in_=xs_d)
        cs_sb = const.tile([IC, 2], f32)
        nc.sync.dma_start(out=cs_sb, in_=cs_d)
        ones1_sb = const.tile([1, 128], bf16)
        nc.vector.memset(ones1_sb, 1.0)
        ones96_sb = const.tile([96, 1], bf16)
        nc.vector.memset(ones96_sb, 1.0)

        q_sb = persist.tile([IC, HW], f32)
        k_sb = persist.tile([IC, HW], f32)
        pc_sb = persist.tile([96, HW], bf16)  # exp(col logits), [g, (w,h)] w-major
        pr_sb = persist.tile([96, HW], bf16)  # exp(row logits), [v, (h,w)] h-major
        rg_sb = persist.tile([96, 96], f32)  # gamma/D, [h, w]
        rgt_sb = persist.tile([96, 96], f32)  # [w, h]

        # ---------- Phase P: dequant, q/k fp32 projection + c, v ----------
        xv = xq_d.rearrange("(cc p) n -> p cc n", p=128)
        vtw = vt_d.rearrange("(q pt p) c -> q p pt c", pt=4, p=128)
        with ExitStack() as ph, tc.tile_pool(name="pstage", bufs=2) as stage, \
                tc.tile_pool(name="qkpsum", bufs=2, space="PSUM") as psqk, \
                tc.tile_pool(name="ppsum", bufs=2, space="PSUM") as psv:
            for nb in range(NB):
                s, e = nb * 512, (nb + 1) * 512
                xqt = stage.tile([128, 4, 512], i8, tag="xq")
                nc.sync.dma_start(out=xqt, in_=xv[:, :, s:e])
                x32 = stage.tile([128, 4, 512], f32, tag="x32")
                for cc in range(4):
                    if (nb + cc) % 2 == 0:
                        nc.vector.tensor_scalar_mul(x32[:, cc, :], xqt[:, cc, :],
                                                    xs_sb[:, cc:cc + 1])
                    else:
                        nc.scalar.activation(x32[:, cc, :], xqt[:, cc, :],
                                             AF.Copy, scale=xs_sb[:, cc:cc + 1])
                xbb = stage.tile([128, 4, 512], bf16, tag="xbb")
                if nb % 2 == 0:
                    nc.scalar.copy(xbb, x32)
                else:
                    nc.gpsimd.tensor_copy(xbb, x32)
                # q/k fp32 projections + int8 correction stream
                cqs = stage.tile([IC, 512], i8, tag="cqs")
                nc.sync.dma_start(out=cqs, in_=cq_d[0:IC, s:e])
                cks = stage.tile([IC, 512], i8, tag="cks")
                nc.sync.dma_start(out=cks, in_=cq_d[IC:2 * IC, s:e])
                pq = psqk.tile([IC, 512], f32, tag="pq")
                pk = psqk.tile([IC, 512], f32, tag="pk")
                for cc in range(4):
                    nc.tensor.matmul(pq, lhsT=wqkT_sb[:, cc, 0:IC],
                                     rhs=x32[:, cc, :],
                                     start=(cc == 0), stop=(cc == 3))
                for cc in range(4):
                    nc.tensor.matmul(pk, lhsT=wqkT_sb[:, cc, IC:2 * IC],
                                     rhs=x32[:, cc, :],
                                     start=(cc == 0), stop=(cc == 3))
                nc.vector.scalar_tensor_tensor(q_sb[:, s:e], cqs,
                                               cs_sb[:, 0:1], pq,
                                               op0=OP.mult, op1=OP.add)
                nc.vector.scalar_tensor_tensor(k_sb[:, s:e], cks,
                                               cs_sb[:, 1:2], pk,
                                               op0=OP.mult, op1=OP.add)
                # v-path
                vstage = stage.tile([128, 4, 512], bf16, tag="vst")
                for pt in range(4):
                    pv = psv.tile([128, 512], f32, tag="pv")
                    for cc in range(4):
                        nc.tensor.matmul(pv, lhsT=xbb[:, cc, pt * 128:(pt + 1) * 128],
                                         rhs=wv_sb[:, cc, :], start=(cc == 0), stop=False)
                    nc.tensor.matmul(pv, lhsT=ones1_sb, rhs=mwvd_sb, start=False, stop=True)
                    if pt % 2 == 0:
                        nc.scalar.copy(vstage[:, pt, :], pv)
                    else:
                        nc.vector.tensor_copy(vstage[:, pt, :], pv)
                nc.sync.dma_start(out=vtw[nb], in_=vstage)

        # ---------------- Phase L: logits, exp, sums ----------------
        kc = k_sb.rearrange("c (g w) -> c g w", w=96)
        qc = q_sb.rearrange("c (g w) -> c g w", w=96)
        with ExitStack() as ph, tc.tile_pool(name="lpsum", bufs=4, space="PSUM") as pse, \
                tc.tile_pool(name="spsum", bufs=2, space="PSUM") as pss, \
                tc.tile_pool(name="sstage", bufs=2) as sst:
            for hg in range(24):
                pe4 = pse.tile([96, 384], f32, tag="pe")
                for hi in range(4):
                    h = hg * 4 + hi
                    sl = slice(hi * 96, (hi + 1) * 96)
                    nc.tensor.matmul(pe4[:, sl], lhsT=k_sb[:, h * 96:(h + 1) * 96],
                                     rhs=q_sb[:, h * 96:(h + 1) * 96],
                                     start=True, stop=True)
                nc.scalar.activation(pr_sb[:, hg * 384:(hg + 1) * 384], pe4, AF.Exp)
            for wg in range(24):
                pe4 = pse.tile([96, 384], f32, tag="pe")
                for wi in range(4):
                    w = wg * 4 + wi
                    sl = slice(wi * 96, (wi + 1) * 96)
                    nc.tensor.matmul(pe4[:, sl], lhsT=kc[:, :, w], rhs=qc[:, :, w],
                                     start=True, stop=False)
                    nc.tensor.matmul(pe4[:, sl], lhsT=ib_sb, rhs=negib_sb,
                                     start=False, stop=True)
                nc.scalar.activation(pc_sb[:, wg * 384:(wg + 1) * 384], pe4, AF.Exp)
            for j in range(NB):
                s, e = j * 512, (j + 1) * 512
                p1 = pss.tile([1, 512], f32, tag="p1")
                nc.tensor.matmul(p1, lhsT=ones96_sb, rhs=pc_sb[:, s:e], start=True, stop=True)
                t1 = sst.tile([1, 512], f32, tag="t1")
                nc.vector.tensor_copy(t1, p1)
                nc.sync.dma_start(out=sc_d[:, s:e], in_=t1)
                p2 = pss.tile([1, 512], f32, tag="p2")
                nc.tensor.matmul(p2, lhsT=ones96_sb, rhs=pr_sb[:, s:e], start=True, stop=True)
                t2 = sst.tile([1, 512], f32, tag="t2")
                nc.scalar.copy(t2, p2)
                nc.sync.dma_start(out=sr_d[:, s:e], in_=t2)

        # ---------------- Phase D: denominators -> Rg, RgT ----------------
        with ExitStack() as ph, tc.tile_pool(name="dsmall", bufs=1) as dsm, \
                tc.tile_pool(name="dpsum", bufs=1, space="PSUM") as dps:
            sct = dsm.tile([96, 96], f32)  # [w, h]
            nc.sync.dma_start(out=sct, in_=sc_d.rearrange("one (w h) -> (one w) h", h=96))
            srt = dsm.tile([96, 96], f32)  # [h, w]
            nc.sync.dma_start(out=srt, in_=sr_d.rearrange("one (h w) -> (one h) w", w=96))
            ptr = dps.tile([96, 96], f32)
            nc.tensor.transpose(ptr, sct, ib_sb)  # -> [h, w]
            d_sb = dsm.tile([96, 96], f32)
            nc.vector.tensor_add(d_sb, ptr, srt)
            r_sb = dsm.tile([96, 96], f32)
            nc.vector.reciprocal(r_sb, d_sb)
            nc.scalar.activation(rg_sb, r_sb, AF.Copy, scale=float(gamma_f))
            ptr2 = dps.tile([96, 96], f32)
            nc.tensor.transpose(ptr2, rg_sb, ib_sb)
            nc.vector.tensor_copy(rgt_sb, ptr2)

        # ------- Phases C+R interleaved: column + row attention -------
        vtc = vt_d.rearrange("(g wg wi) c -> wg g wi c", wg=24, wi=4)
        ucw = uc_d.rearrange("(h wg wi) c -> wg h wi c", wg=24, wi=4)
        vtr = vt_d.rearrange("(hg hi v) c -> hg v hi c", hg=24, hi=4)
        urw = ur_d.rearrange("(hg hi w) c -> hg w hi c", hg=24, hi=4)
        with ExitStack() as ph, tc.tile_pool(name="crstage", bufs=4) as cst, \
                tc.tile_pool(name="cpsum", bufs=3, space="PSUM") as psu, \
                tc.tile_pool(name="rpsum", bufs=3, space="PSUM") as psr:
            for grp in range(24):
                wg = grp
                vc = cst.tile([96, 4, C], bf16, tag="vc")
                nc.sync.dma_start(out=vc, in_=vtc[wg])
                uc = cst.tile([96, 4, C], bf16, tag="uc")
                for wi in range(4):
                    w = wg * 4 + wi
                    pu = psu.tile([96, C], f32, tag="pu")
                    nc.tensor.matmul(pu, lhsT=pc_sb[:, w * 96:(w + 1) * 96],
                                     rhs=vc[:, wi, :], start=True, stop=True)
                    if w % 2 == 0:
                        nc.scalar.activation(uc[:, wi, :], pu, AF.Copy,
                                             scale=rg_sb[:, w:w + 1])
                    else:
                        nc.vector.tensor_scalar_mul(uc[:, wi, :], pu, rg_sb[:, w:w + 1])
                nc.sync.dma_start(out=ucw[wg], in_=uc)
                hg = grp
                vr = cst.tile([96, 4, C], bf16, tag="vr")
                nc.sync.dma_start(out=vr, in_=vtr[hg])
                ur = cst.tile([96, 4, C], bf16, tag="ur")
                for hi in range(4):
                    h = hg * 4 + hi
                    pu = psr.tile([96, C], f32, tag="pur")
                    nc.tensor.matmul(pu, lhsT=pr_sb[:, h * 96:(h + 1) * 96],
                                     rhs=vr[:, hi, :], start=True, stop=True)
                    if h % 2 == 0:
                        nc.scalar.activation(ur[:, hi, :], pu, AF.Copy,
                                             scale=rgt_sb[:, h:h + 1])
                    else:
                        nc.vector.tensor_scalar_mul(ur[:, hi, :], pu, rgt_sb[:, h:h + 1])
                nc.sync.dma_start(out=urw[hg], in_=ur)

        # ------- Phase F: delta = uc+ur, int6 quantization + packing -------
        # delta carries the gamma/D scaling; residual add happens on host.
        # Per (channel, 512px-block): qv = round(delta * 30.5/amax) + 32 in
        # [1,63]; groups of 4 pixels pack into 3 bytes (little-endian 6-bit).
        with ExitStack() as ph, tc.tile_pool(name="fstage", bufs=3) as fst, \
                tc.tile_pool(name="fwork", bufs=1) as fwk, \
                tc.tile_pool(name="fpk", bufs=2) as fpkp, \
                tc.tile_pool(name="fsball", bufs=1) as fsb:
            for cc in range(4):
                cs = slice(cc * 128, (cc + 1) * 128)
                sball = fsb.tile([128, HW], bf16, tag="sball")
                for hb in range(6):
                    r0 = hb * 1536
                    uct = fst.tile([128, 1536], bf16, tag="uct")
                    nc.sync.dma_start(out=uct, in_=uc_d[r0:r0 + 1536, cs], transpose=True)
                    urt = fst.tile([128, 1536], bf16, tag="urt")
                    nc.sync.dma_start(out=urt, in_=ur_d[r0:r0 + 1536, cs], transpose=True)
                    if (cc + hb) % 2 == 0:
                        nc.gpsimd.tensor_add(sball[:, r0:r0 + 1536], uct, urt)
                    else:
                        nc.vector.tensor_add(sball[:, r0:r0 + 1536], uct, urt)
                amax = fwk.tile([128, NB], f32, tag="amax")
                for j in range(NB):
                    nc.vector.tensor_reduce(amax[:, j:j + 1], sball[:, j * 512:(j + 1) * 512],
                                            axis=mybir.AxisListType.X,
                                            op=mybir.AluOpType.max,
                                            apply_absolute_value=True)
                nc.sync.dma_start(out=outs_d[cs, :], in_=amax)
                rinv = fwk.tile([128, NB], f32, tag="rinv")
                nc.vector.reciprocal(rinv, amax)
                rs = fwk.tile([128, NB], f32, tag="rs")
                nc.scalar.activation(rs, rinv, AF.Copy, scale=QMARGIN)
                qv = fwk.tile([128, HW], u8, tag="qv")
                for j in range(NB):
                    nc.vector.tensor_scalar(qv[:, j * 512:(j + 1) * 512],
                                            sball[:, j * 512:(j + 1) * 512],
                                            rs[:, j:j + 1], 32.0,
                                            op0=OP.mult, op1=OP.add)
                # pack: w24 = v0 | v1<<6 | v2<<12 | v3<<18 -> 3 bytes
                g4 = qv.rearrange("p (n four) -> p n four", four=4)
                v0, v1 = g4[:, :, 0], g4[:, :, 1]
                v2, v3 = g4[:, :, 2], g4[:, :, 3]
                pk = fpkp.tile([128, HW // 4, 3], u8, tag="pk")
                t1 = fwk.tile([128, HW // 4], u8, tag="t1")
                nc.vector.tensor_scalar(t1, v1, 3, 6, op0=OP.bitwise_and,
                                        op1=OP.logical_shift_left)
                nc.vector.tensor_tensor(pk[:, :, 0], v0, t1, op=OP.bitwise_or)
                t2 = fwk.tile([128, HW // 4], u8, tag="t2")
                nc.vector.tensor_scalar(t2, v1, 2, None, op0=OP.logical_shift_right)
                t3 = fwk.tile([128, HW // 4], u8, tag="t3")
                nc.vector.tensor_scalar(t3, v2, 15, 4, op0=OP.bitwise_and,
                                        op1=OP.logical_shift_left)
                nc.vector.tensor_tensor(pk[:, :, 1], t2, t3, op=OP.bitwise_or)
                t4 = fwk.tile([128, HW // 4], u8, tag="t4")
                nc.vector.tensor_scalar(t4, v2, 4, None, op0=OP.logical_shift_right)
                t5 = fwk.tile([128, HW // 4], u8, tag="t5")
                nc.vector.tensor_scalar(t5, v3, 2, None, op0=OP.logical_shift_left)
                nc.vector.tensor_tensor(pk[:, :, 2], t4, t5, op=OP.bitwise_or)
                nc.sync.dma_start(out=outp_d[cs, :],
                                  in_=pk.rearrange("p n three -> p (n three)"))

    nc.compile()
    return nc


NGROUPS = int(__import__("os").environ.get("KERNEL_NGROUPS", "2"))


def _make_runner(gamma_f: float, ngroups: int = NGROUPS):
    """Build the Bass module once and wrap it in cached jitted dispatchers
    (the axon run_bass_kernel_spmd path, minus the per-call retrace, minus
    the host-side zero-output upload). The 8 cores are split into `ngroups`
    independent dispatch groups so a later group's upload/exec overlaps an
    earlier group's download through the shared tunnel."""
    import jax
    import jax.numpy as jnp
    from jax.sharding import Mesh, PartitionSpec, NamedSharding
    try:
        from jax.experimental.shard_map import shard_map
    except ImportError:
        from jax.shard_map import shard_map
    from concourse import bass2jax, mybir
    from concourse.bass2jax import _bass_exec_p, install_neuronx_cc_hook

    nc = _build(gamma_f)
    install_neuronx_cc_hook()
    if nc.dbg_addr is not None and nc.dbg_callbacks:
        raise RuntimeError("dbg callbacks unsupported in cached dispatch")

    partition_name = nc.partition_id_tensor.name if nc.partition_id_tensor else None
    in_names, out_names, out_avals = [], [], []
    for alloc in nc.m.functions[0].allocations:
        if not isinstance(alloc, mybir.MemoryLocationSet):
            continue
        name = alloc.memorylocations[0].name
        if alloc.kind == "ExternalInput":
            if name != partition_name:
                in_names.append(name)
        elif alloc.kind == "ExternalOutput":
            out_names.append(name)
            out_avals.append(jax.core.ShapedArray(
                tuple(alloc.tensor_shape), mybir.dt.np(alloc.dtype)))
    n_params = len(in_names)
    n_outs = len(out_names)
    bind_in_names = tuple(in_names + out_names
                          + ([partition_name] if partition_name else []))

    def _body(*args):
        operands = list(args)
        if partition_name is not None:
            operands.append(bass2jax.partition_id_tensor())
        outs = _bass_exec_p.bind(
            *operands,
            out_avals=tuple(out_avals),
            in_names=bind_in_names,
            out_names=tuple(out_names),
            lowering_input_output_aliases=(),
            sim_require_finite=True,
            sim_require_nnan=True,
            nc=nc,
        )
        return tuple(outs)

    devices = jax.devices()[:NCORES]
    assert len(devices) == NCORES, f"need {NCORES} devices, have {len(jax.devices())}"
    assert NCORES % ngroups == 0
    gsize = NCORES // ngroups
    in_specs = (PartitionSpec("core"),) * (n_params + n_outs)
    out_specs = (PartitionSpec("core"),) * n_outs
    donate = tuple(range(n_params, n_params + n_outs))
    groups = []
    for gi in range(ngroups):
        mesh = Mesh(np.asarray(devices[gi * gsize:(gi + 1) * gsize]), ("core",))
        nshard = NamedSharding(mesh, PartitionSpec("core"))
        fn = jax.jit(
            shard_map(_body, mesh=mesh, in_specs=in_specs, out_specs=out_specs,
                      check_rep=False),
            donate_argnums=donate,
            keep_unused=True,
        )
        # Donated zero output buffers, created on-device (no host upload).
        zero_fns = [
            jax.jit(
                (lambda shape, dt: (lambda: jnp.zeros(shape, dt)))(
                    (gsize * av.shape[0],) + tuple(av.shape[1:]), av.dtype),
                out_shardings=nshard)
            for av in out_avals
        ]
        groups.append(dict(fn=fn, zero_fns=zero_fns, nshard=nshard))
    return dict(nc=nc, groups=groups, gsize=gsize,
                in_names=in_names, out_names=out_names)


_cache: dict = {}
_bufs: dict = {}

try:
    import numba

    @numba.njit(cache=True, fastmath=True, nogil=True)
    def _nb_quant_resid(xb, q8, srow, r32):
        # q8 = rint(xb * 127/amax_row); srow = amax_row/127; r32 = xb - q8*srow
        for c in range(xb.shape[0]):
            row = xb[c]
            m = 1e-20
            for p in range(row.shape[0]):
                a = abs(row[p])
                if a > m:
                    m = a
            s = 127.0 / m
            si = m / 127.0
            qrow = q8[c]
            rrow = r32[c]
            for p in range(row.shape[0]):
                qp = np.int8(np.rint(row[p] * s))
                qrow[p] = qp
                rrow[p] = row[p] - qp * si
            srow[c] = si

    @numba.njit(cache=True, fastmath=True, nogil=True)
    def _nb_quant_c(cb, q8, cs2):
        # cb [128, HW] -> int8 rows; cs2 [64, 2]: col0 = q rows, col1 = k rows
        n = cb.shape[0] // 2
        for c in range(cb.shape[0]):
            row = cb[c]
            m = 1e-20
            for p in range(row.shape[0]):
                a = abs(row[p])
                if a > m:
                    m = a
            s = 127.0 / m
            qrow = q8[c]
            for p in range(row.shape[0]):
                qrow[p] = np.int8(np.rint(row[p] * s))
            if c < n:
                cs2[c, 0] = m / 127.0
            else:
                cs2[c - n, 1] = m / 127.0

    @numba.njit(cache=True, fastmath=True, nogil=True)
    def _nb_unpack_dequant(acc, pk, am18):
        # acc [C, HW] += unpack6(pk [C, PACKW]) scaled by am18 [C, 18]/30.5
        ngrp = pk.shape[1] // 3
        for c in range(acc.shape[0]):
            prow = pk[c]
            arow = acc[c]
            for j in range(18):
                step = am18[c, j] * (1.0 / 30.5)
                g0 = j * 128  # 128 groups of 4 px per 512-block
                for gg in range(128):
                    gi = g0 + gg
                    b0 = np.int32(prow[3 * gi])
                    b1 = np.int32(prow[3 * gi + 1])
                    b2 = np.int32(prow[3 * gi + 2])
                    p0 = gi * 4
                    arow[p0] += ((b0 & 63) - 32) * step
                    arow[p0 + 1] += (((b0 >> 6) | ((b1 & 15) << 2)) - 32) * step
                    arow[p0 + 2] += (((b1 >> 4) | ((b2 & 3) << 4)) - 32) * step
                    arow[p0 + 3] += ((b2 >> 2) - 32) * step
except Exception:  # numba unavailable: numpy fallbacks below
    _nb_quant_resid = None
    _nb_quant_c = None
    _nb_unpack_dequant = None


def _np_quant_resid(xb, q8, srow, r32):
    amax = np.maximum(np.abs(xb).max(axis=1), 1e-20)
    si = amax * (1.0 / 127.0)
    np.rint(xb * (127.0 / amax)[:, None], out=r32)
    q8[:] = r32
    np.multiply(q8, si[:, None], out=r32)
    np.subtract(xb, r32, out=r32)
    srow[:] = si


def _np_quant_c(cb, q8, cs2):
    n = cb.shape[0] // 2
    amax = np.maximum(np.abs(cb).max(axis=1), 1e-20)
    q8[:] = np.rint(cb * (127.0 / amax)[:, None])
    cs2[:, 0] = amax[:n] * (1.0 / 127.0)
    cs2[:, 1] = amax[n:] * (1.0 / 127.0)


def _np_unpack_dequant(acc, pk, am18):
    b = pk.reshape(acc.shape[0], -1, 3).astype(np.int32)
    v = np.empty((acc.shape[0], b.shape[1], 4), np.int32)
    v[:, :, 0] = b[:, :, 0] & 63
    v[:, :, 1] = (b[:, :, 0] >> 6) | ((b[:, :, 1] & 15) << 2)
    v[:, :, 2] = (b[:, :, 1] >> 4) | ((b[:, :, 2] & 3) << 4)
    v[:, :, 3] = b[:, :, 2] >> 2
    vals = (v - 32).reshape(acc.shape[0], 18, 512).astype(np.float32)
    vals *= (am18 * (1.0 / 30.5))[:, :, None]
    acc += vals.reshape(acc.shape)


def _prep_shared(Wq, bq, Wk, bk, Wv, bv, delta):
    Wq = np.asarray(Wq, np.float32)
    Wk = np.asarray(Wk, np.float32)
    Wv = np.asarray(Wv, np.float32)
    Wqk = np.ascontiguousarray(np.vstack([Wq, Wk]))  # (2*IC, C)
    dev = dict(
        wqkT=np.ascontiguousarray(Wqk.T).astype(np.float32).reshape(4, 128, 2 * IC),
        wvT=np.ascontiguousarray(Wv.T).astype(BF).reshape(4, 128, C),
        mwvd=(-(Wv @ delta)).astype(BF).reshape(1, C),
        ib=np.eye(96, dtype=np.float32),
        negib=np.eye(96, dtype=np.float32) * -1e30,
    )
    bqk = np.concatenate([np.asarray(bq, np.float32) - Wq @ delta,
                          np.asarray(bk, np.float32) - Wk @ delta])
    return dev, Wqk, bqk


def _getbuf(name, shape, dtype):
    b = _bufs.get(name)
    if b is None or b.shape != shape or b.dtype != dtype:
        b = _bufs[name] = np.empty(shape, dtype)
    return b


_PROF = bool(__import__("os").environ.get("KERNEL_PROF"))


def _host_prep(b, x32, delta, Wqk, bqk, xd32, xq8, cq8, xs_np, cs_np, r32, c32):
    np.add(x32[b], delta[:, None], out=xd32[b])
    xb = xd32[b]
    srow = np.empty(C, np.float32)
    if _nb_quant_resid is not None:
        _nb_quant_resid(xb, xq8[b], srow, r32)
    else:
        _np_quant_resid(xb, xq8[b], srow, r32)
    xs_np[b] = srow.reshape(4, 128).T
    np.matmul(Wqk, r32, out=c32)
    c32 += bqk[:, None]
    if _nb_quant_c is not None:
        _nb_quant_c(c32, cq8[b], cs_np[b])
    else:
        _np_quant_c(c32, cq8[b], cs_np[b])


def _run_fast(runner, x32, delta, shared, Wqk, bqk, B):
    import jax
    import time as _t
    _t0 = _t.time()
    _mark = (lambda m: print(f"  [{_t.time()-_t0:7.3f}] {m}", flush=True)) \
        if _PROF else (lambda m: None)
    groups = runner["groups"]
    gsize = runner["gsize"]
    xd32 = np.empty((B, C, HW), np.float32)  # x + gamma*bv: residual base
    xq8 = _getbuf("xq8", (B, C, HW), np.int8)
    cq8 = _getbuf("cq8", (B, 2 * IC, HW), np.int8)
    xs_np = _getbuf("xs", (B, 128, 4), np.float32)
    cs_np = _getbuf("cs", (B, IC, 2), np.float32)
    r32 = _getbuf("r32", (C, HW), np.float32)
    c32 = _getbuf("c32", (2 * IC, HW), np.float32)
    wc = runner.get("wcache")
    fresh_w = not (wc is not None
                   and all(np.array_equal(shared[n], wc[0][n]) for n in shared))
    if fresh_w:
        dev_w_g = []
    pi = runner["out_names"].index("outp")
    si = runner["out_names"].index("outs")

    devices = jax.devices()[:NCORES]

    disp = []
    with ThreadPoolExecutor(NCORES) as ex:
        for gi, gr in enumerate(groups):
            b0, b1 = gi * gsize, (gi + 1) * gsize
            # stream each batch's planes up as soon as they're ready
            xparts, cparts = [], []
            for b in range(b0, b1):
                _host_prep(b, x32, delta, Wqk, bqk, xd32, xq8, cq8,
                           xs_np, cs_np, r32, c32)
                xparts.append(jax.device_put(xq8[b], devices[b]))
                cparts.append(jax.device_put(cq8[b], devices[b]))
            xg = jax.make_array_from_single_device_arrays(
                (gsize * C, HW), gr["nshard"], xparts)
            cg = jax.make_array_from_single_device_arrays(
                (gsize * 2 * IC, HW), gr["nshard"], cparts)
            _mark(f"g{gi} staged")
            sg = jax.device_put(xs_np[b0:b1].reshape(gsize * 128, 4),
                                gr["nshard"])
            csg = jax.device_put(cs_np[b0:b1].reshape(gsize * IC, 2),
                                 gr["nshard"])
            per_call = {"xq": xg, "cq": cg, "xs": sg, "cs": csg}
            if fresh_w:
                dev_w = {n: jax.device_put(np.concatenate([w] * gsize, axis=0),
                                           gr["nshard"])
                         for n, w in shared.items()}
                dev_w_g.append(dev_w)
            else:
                dev_w = wc[1][gi]
            args = [per_call.get(name) if name in per_call else dev_w[name]
                    for name in runner["in_names"]]
            zeros = gr.pop("next_zeros", None) or [zf() for zf in gr["zero_fns"]]
            disp.append(gr["fn"](*args, *zeros))
            _mark(f"g{gi} dispatched")
        if fresh_w:
            runner["wcache"] = ({n: np.copy(w) for n, w in shared.items()},
                                dev_w_g)
        for gi, gr in enumerate(groups):
            # prefetch next call's donated zero buffers; overlaps exec/fetch
            gr["next_zeros"] = [zf() for zf in gr["zero_fns"]]

        jobs = []
        for gi, out_arrs in enumerate(disp):
            jobs.extend((gi, s) for s in out_arrs[pi].addressable_shards)

        def fetch(arg):
            gi, s = arg
            pk = np.asarray(s.data)  # (C, PACKW) u8; blocks until device done
            snp = np.asarray(disp[gi][si]).reshape(gsize, C, NB)
            b = gi * gsize + (s.index[0].start or 0) // C
            am18 = snp[(s.index[0].start or 0) // C]
            if _nb_unpack_dequant is not None:
                _nb_unpack_dequant(xd32[b], pk, am18)
            else:
                _np_unpack_dequant(xd32[b], pk, am18)

        list(ex.map(fetch, jobs))
        _mark("all fetched")
    return xd32


def _run_fallback(nc, x32, delta, shared, Wqk, bqk, B):
    from concourse.bass_utils import run_bass_kernel_spmd
    xd32 = x32 + delta[None, :, None]
    in_maps = []
    for b in range(B):
        xb = xd32[b]
        xq8 = np.empty((C, HW), np.int8)
        cq8 = np.empty((2 * IC, HW), np.int8)
        srow = np.empty(C, np.float32)
        cs2 = np.empty((IC, 2), np.float32)
        r32 = np.empty((C, HW), np.float32)
        _np_quant_resid(xb, xq8, srow, r32)
        c32 = Wqk @ r32 + bqk[:, None]
        _np_quant_c(c32, cq8, cs2)
        in_maps.append(dict(
            shared,
            xq=xq8,
            xs=np.ascontiguousarray(srow.reshape(4, 128).T),
            cq=cq8,
            cs=cs2,
        ))
    res = run_bass_kernel_spmd(nc, in_maps, core_ids=list(range(B)))
    for b in range(B):
        _np_unpack_dequant(xd32[b], res.results[b]["outp"],
                           res.results[b]["outs"])
    return xd32


def kernel(x, Wq, bq, Wk, bk, Wv, bv, gamma):
    x = np.asarray(x)
    B = x.shape[0]
    assert B == NCORES, f"expected B={NCORES}, got {B}"
    g = float(np.asarray(gamma).reshape(-1)[0])
    delta = (g * np.asarray(bv, np.float64)).astype(np.float32)
    x32 = np.asarray(x, np.float32).reshape(B, C, HW)
    shared, Wqk, bqk = _prep_shared(Wq, bq, Wk, bk, Wv, bv, delta)

    key = round(g, 9)
    if key not in _cache:
        _cache[key] = _make_runner(g)
    runner = _cache[key]

    globals()["_last_exec_ns"] = None
    globals()["_last_trace"] = None
    try:
        res = _run_fast(runner, x32, delta, shared, Wqk, bqk, B)
    except Exception:
        import os, sys, time, traceback
        traceback.print_exc()
        if os.environ.get("KERNEL_NO_FALLBACK"):
            raise
        # A wedged NeuronCore (NRT_EXEC_UNIT_UNRECOVERABLE) persists for the
        # life of the PJRT client: tear the backend down, rebuild the runner
        # (terminal-side reconnect resets the cores), and retry once.
        print("kernel: fast dispatch failed; resetting backend", file=sys.stderr)
        try:
            import jax._src.xla_bridge as _xb
            _xb._clear_backends()
            import jax
            jax.clear_caches()
            time.sleep(2.0)
            _cache.clear()
            _cache[key] = runner = _make_runner(g)
            res = _run_fast(runner, x32, delta, shared, Wqk, bqk, B)
        except Exception:
            traceback.print_exc()
            print("kernel: retry failed; final fallback", file=sys.stderr)
            res = _run_fallback(runner["nc"], x32, delta, shared, Wqk, bqk, B)
    return res.reshape(B, C, H, W)


# revision 8
# speedup vs baseline: 1.5695x; 1.0585x over previous
"""CrissCrossAttention Trainium2 kernel.

Per-core: one batch b of x [C=512, HW=9216] (h-major pixels, p = h*96+w).

Math (reference):
  q = Wq x + bq ; k = Wk x + bk ; v = Wv x + bv        (1x1 convs)
  E_col[g,h] per w = sum_c k[c,g,w] q[c,h,w]  (diag g==h masked -inf)
  E_row[v,w] per h                                      (row logits)
  attn = softmax over concat(H' + W') per dest pixel
  out = gamma*(out_h + out_w) + x

The end-to-end time in this axon-tunneled setup is dominated by the
host<->device wire (~45-90MB/s shared, up+down combined), so the design
minimizes bytes on the wire and overlaps host work / upload / exec /
download.  Wire budget per batch: 5.9MB up + 3.6MB down (vs 11.8MB in the
naive int8-x + fp16-qk + int8-out layout).

Host (single CPU core):
  - x' = x + gamma*bv (residual shift folding bv; v-path correction row
    -Wv(gamma*bv) added on device via K=1 matmul; bq/bk adjusted inside the
    correction stream).
  - x' shipped as per-channel int8 [C, HW] + fp32 scales.
  - q/k are NOT shipped.  The device computes qk = Wqk @ xhat in fp32 (PE),
    where xhat is the int8-dequantized x'.  The host ships a small int8
    correction stream cq = quant(Wqk @ (x' - xhat) + bqk) [2*IC, HW]
    (1.2MB).  Host and device both evaluate Wqk @ xhat in fp32, so adding
    the correction reconstructs q/k to ~fp32 accuracy at 1/2 the bytes of
    fp16 q/k shipping.  Host pays one [128,512]x[512,9216] sgemm on the
    quantization residual (same flops as shipping q/k directly would).
  - residual add and the output-delta dequant happen on host in fp32.

Device (per core, Tile framework):
  - Phase P: dequant int8 x' -> fp32 x32 and bf16 xbb; q/k = fp32 PE
    matmuls of wqkT x32 + int8 correction; v = Wv xbb via bf16 matmuls.
  - P = exp(logits) unnormalized bf16 (values up to e^40 overflow fp16);
    denominators D = colsum + rowsum via ones-matmuls; Rg = gamma/D.
  - U_colT(w) / U_rowT(h) -> [96, C] bf16 scratch in DRAM; final pass
    DMA-transposes them back, sums to delta = gamma*(out_h+out_w), and
    emits delta as int6 (4 values packed into 3 bytes) with per-channel
    per-512px-block fp32 amax [C, 18].

Dispatch: replicates run_bass_kernel_spmd's axon path (shard_map +
_bass_exec_p custom call) but builds the jitted executables ONCE and caches
them; donated zero output buffers are created on-device (no host upload);
the 8 cores run as 2 groups of 4 so the second group's upload/exec overlaps
the first group's download; all shard fetches share one thread pool so the
wire stays saturated. On a wedged NeuronCore the backend is torn down,
rebuilt, and the call retried before falling back to run_bass_kernel_spmd.
"""

import numpy as np
import ml_dtypes
from concurrent.futures import ThreadPoolExecutor

C, IC, H, W = 512, 64, 96, 96
HW = H * W  # 9216
NB = 18  # 512-wide pixel blocks
NCORES = 8
BF = ml_dtypes.bfloat16
QMARGIN = 30.5  # int6 quant margin (|q| <= 31)
PACKW = HW // 4 * 3  # 6912 packed bytes per channel row (int6 out, int6 c)
XPACKW = HW // 8 * 7  # 8064 packed bytes per channel row (int7 x)


def _build(gamma_f: float):
    from contextlib import ExitStack
    import concourse.bass as bass
    import concourse.bacc as bacc
    import concourse.tile as tile
    from concourse import mybir

    f32 = mybir.dt.float32
    bf16 = mybir.dt.bfloat16
    i8 = mybir.dt.int8
    u8 = mybir.dt.uint8
    AF = mybir.ActivationFunctionType
    OP = mybir.AluOpType

    nc = bacc.Bacc("TRN2", target_bir_lowering=False, debug=False)

    xp_d = nc.dram_tensor("xp", [C, XPACKW], u8, kind="ExternalInput").ap()
    xs_d = nc.dram_tensor("xs", [128, 4], f32, kind="ExternalInput").ap()
    cp_d = nc.dram_tensor("cp", [2 * IC, PACKW], u8, kind="ExternalInput").ap()
    cs_d = nc.dram_tensor("cs", [IC, 2], f32, kind="ExternalInput").ap()
    csn_d = nc.dram_tensor("csn", [2, 512], f32, kind="ExternalInput").ap()
    wqkT_d = nc.dram_tensor("wqkT", [4, 128, 2 * IC], f32, kind="ExternalInput").ap()
    wv_d = nc.dram_tensor("wvT", [4, 128, C], bf16, kind="ExternalInput").ap()
    mwvd_d = nc.dram_tensor("mwvd", [1, C], bf16, kind="ExternalInput").ap()
    ib_d = nc.dram_tensor("ib", [96, 96], f32, kind="ExternalInput").ap()
    negib_d = nc.dram_tensor("negib", [96, 96], f32, kind="ExternalInput").ap()
    outp_d = nc.dram_tensor("outp", [C, PACKW], u8, kind="ExternalOutput").ap()
    outs_d = nc.dram_tensor("outs", [C, NB], f32, kind="ExternalOutput").ap()

    vt_d = nc.dram_tensor("vt_scratch", [HW, C], bf16, kind="Internal").ap()
    uc_d = nc.dram_tensor("uc_scratch", [HW, C], bf16, kind="Internal").ap()
    ur_d = nc.dram_tensor("ur_scratch", [HW, C], bf16, kind="Internal").ap()
    sc_d = nc.dram_tensor("sc_scratch", [1, HW], f32, kind="Internal").ap()
    sr_d = nc.dram_tensor("sr_scratch", [1, HW], f32, kind="Internal").ap()

    with tile.TileContext(nc) as tc, ExitStack() as top:
        const = top.enter_context(tc.tile_pool(name="const", bufs=1))
        persist = top.enter_context(tc.tile_pool(name="persist", bufs=1))

        wv_sb = const.tile([128, 4, C], bf16)
        nc.sync.dma_start(out=wv_sb, in_=wv_d.rearrange("c p m -> p c m"))
        wqkT_sb = const.tile([128, 4, 2 * IC], f32)
        nc.sync.dma_start(out=wqkT_sb, in_=wqkT_d.rearrange("c p m -> p c m"))
        mwvd_sb = const.tile([1, C], bf16)
        nc.sync.dma_start(out=mwvd_sb, in_=mwvd_d)
        ib_sb = const.tile([96, 96], f32)
        nc.sync.dma_start(out=ib_sb, in_=ib_d)
        negib_sb = const.tile([96, 96], f32)
        nc.sync.dma_start(out=negib_sb, in_=negib_d)
        xs_sb = const.tile([128, 4], f32)
        nc.sync.dma_start(out=xs_sb, in_=xs_d)
        cs_sb = const.tile([IC, 2], f32)
        nc.sync.dma_start(out=cs_sb, in_=cs_d)
        csnq_sb = const.tile([1, 512], f32)  # -32*cs_q in cols 0:IC
        nc.sync.dma_start(out=csnq_sb, in_=csn_d[0:1, :])
        csnk_sb = const.tile([1, 512], f32)  # -32*cs_k in cols 0:IC
        nc.sync.dma_start(out=csnk_sb, in_=csn_d[1:2, :])
        ones1_sb = const.tile([1, 128], bf16)
        nc.vector.memset(ones1_sb, 1.0)
        ones96_sb = const.tile([96, 1], bf16)
        nc.vector.memset(ones96_sb, 1.0)
        onesf_sb = const.tile([1, 512], f32)
        nc.vector.memset(onesf_sb, 1.0)
        xoff_sb = const.tile([128, 4], f32)  # -64 * step, for int7 decode
        nc.vector.tensor_scalar_mul(xoff_sb, xs_sb, -64.0)

        q_sb = persist.tile([IC, HW], f32)
        k_sb = persist.tile([IC, HW], f32)
        pc_sb = persist.tile([96, HW], bf16)  # exp(col logits), [g, (w,h)] w-major
        pr_sb = persist.tile([96, HW], bf16)  # exp(row logits), [v, (h,w)] h-major
        rg_sb = persist.tile([96, 96], f32)  # gamma/D, [h, w]
        rgt_sb = persist.tile([96, 96], f32)  # [w, h]

        # ---------- Phase P: unpack+dequant, q/k fp32 projection + c, v ----
        xv = xp_d.rearrange("(cc p) n -> p cc n", p=128)
        vtw = vt_d.rearrange("(q pt p) c -> q p pt c", pt=4, p=128)
        with ExitStack() as ph, tc.tile_pool(name="pstage", bufs=2) as stage, \
                tc.tile_pool(name="qkpsum", bufs=2, space="PSUM") as psqk, \
                tc.tile_pool(name="ppsum", bufs=2, space="PSUM") as psv:
            for nb in range(NB):
                s, e = nb * 512, (nb + 1) * 512
                xpt = stage.tile([128, 4, 448], u8, tag="xp")
                nc.sync.dma_start(out=xpt, in_=xv[:, :, nb * 448:(nb + 1) * 448])
                # int7 unpack: 7 bytes -> 8 values (LE 56-bit words)
                bl = xpt.rearrange("p cc (n seven) -> p cc n seven", seven=7)
                xu = stage.tile([128, 4, 512], u8, tag="xu")
                vl = xu.rearrange("p cc (n eight) -> p cc n eight", eight=8)
                ta = stage.tile([128, 4, 64], u8, tag="ta")
                tb = stage.tile([128, 4, 64], u8, tag="tb")
                nc.vector.tensor_scalar(vl[:, :, :, 0], bl[:, :, :, 0], 127, None,
                                        op0=OP.bitwise_and)
                for i in range(1, 7):
                    # v_i = (b_{i-1} >> (8-i)) | ((b_i & (2^(7-i)-1)) << i)
                    nc.vector.tensor_scalar(ta, bl[:, :, :, i - 1], 8 - i, None,
                                            op0=OP.logical_shift_right)
                    nc.vector.tensor_scalar(tb, bl[:, :, :, i], (1 << (7 - i)) - 1,
                                            i, op0=OP.bitwise_and,
                                            op1=OP.logical_shift_left)
                    nc.vector.tensor_tensor(vl[:, :, :, i], ta, tb, op=OP.bitwise_or)
                nc.vector.tensor_scalar(vl[:, :, :, 7], bl[:, :, :, 6], 1, None,
                                        op0=OP.logical_shift_right)
                # dequant: x32 = xu*step - 64*step
                x32 = stage.tile([128, 4, 512], f32, tag="x32")
                for cc in range(4):
                    if (nb + cc) % 2 == 0:
                        nc.vector.tensor_scalar(x32[:, cc, :], xu[:, cc, :],
                                                xs_sb[:, cc:cc + 1],
                                                xoff_sb[:, cc:cc + 1],
                                                op0=OP.mult, op1=OP.add)
                    else:
                        nc.scalar.activation(x32[:, cc, :], xu[:, cc, :],
                                             AF.Identity,
                                             bias=xoff_sb[:, cc:cc + 1],
                                             scale=xs_sb[:, cc:cc + 1])
                xbb = stage.tile([128, 4, 512], bf16, tag="xbb")
                if nb % 2 == 0:
                    nc.scalar.copy(xbb, x32)
                else:
                    nc.gpsimd.tensor_copy(xbb, x32)
                # int6 correction stream: 3 bytes -> 4 values, offset 32
                cqs = stage.tile([IC, 384], u8, tag="cqs")
                nc.sync.dma_start(out=cqs, in_=cp_d[0:IC, nb * 384:(nb + 1) * 384])
                cks = stage.tile([IC, 384], u8, tag="cks")
                nc.sync.dma_start(out=cks, in_=cp_d[IC:2 * IC, nb * 384:(nb + 1) * 384])
                cuq = stage.tile([IC, 512], u8, tag="cuq")
                cuk = stage.tile([IC, 512], u8, tag="cuk")
                tc1 = stage.tile([IC, 128], u8, tag="tc1")
                tc2 = stage.tile([IC, 128], u8, tag="tc2")
                for csrc, cdst in ((cqs, cuq), (cks, cuk)):
                    b3 = csrc.rearrange("p (n three) -> p n three", three=3)
                    v4 = cdst.rearrange("p (n four) -> p n four", four=4)
                    nc.vector.tensor_scalar(v4[:, :, 0], b3[:, :, 0], 63, None,
                                            op0=OP.bitwise_and)
                    nc.vector.tensor_scalar(tc1, b3[:, :, 0], 6, None,
                                            op0=OP.logical_shift_right)
                    nc.vector.tensor_scalar(tc2, b3[:, :, 1], 15, 2,
                                            op0=OP.bitwise_and,
                                            op1=OP.logical_shift_left)
                    nc.vector.tensor_tensor(v4[:, :, 1], tc1, tc2, op=OP.bitwise_or)
                    nc.vector.tensor_scalar(tc1, b3[:, :, 1], 4, None,
                                            op0=OP.logical_shift_right)
                    nc.vector.tensor_scalar(tc2, b3[:, :, 2], 3, 4,
                                            op0=OP.bitwise_and,
                                            op1=OP.logical_shift_left)
                    nc.vector.tensor_tensor(v4[:, :, 2], tc1, tc2, op=OP.bitwise_or)
                    nc.vector.tensor_scalar(v4[:, :, 3], b3[:, :, 2], 2, None,
                                            op0=OP.logical_shift_right)
                pq = psqk.tile([IC, 512], f32, tag="pq")
                pk = psqk.tile([IC, 512], f32, tag="pk")
                for cc in range(4):
                    nc.tensor.matmul(pq, lhsT=wqkT_sb[:, cc, 0:IC],
                                     rhs=x32[:, cc, :],
                                     start=(cc == 0), stop=False)
                nc.tensor.matmul(pq, lhsT=csnq_sb[:, 0:IC], rhs=onesf_sb,
                                 start=False, stop=True)
                for cc in range(4):
                    nc.tensor.matmul(pk, lhsT=wqkT_sb[:, cc, IC:2 * IC],
                                     rhs=x32[:, cc, :],
                                     start=(cc == 0), stop=False)
                nc.tensor.matmul(pk, lhsT=csnk_sb[:, 0:IC], rhs=onesf_sb,
                                 start=False, stop=True)
                nc.vector.scalar_tensor_tensor(q_sb[:, s:e], cuq,
                                               cs_sb[:, 0:1], pq,
                                               op0=OP.mult, op1=OP.add)
                nc.vector.scalar_tensor_tensor(k_sb[:, s:e], cuk,
                                               cs_sb[:, 1:2], pk,
                                               op0=OP.mult, op1=OP.add)
                # v-path
                vstage = stage.tile([128, 4, 512], bf16, tag="vst")
                for pt in range(4):
                    pv = psv.tile([128, 512], f32, tag="pv")
                    for cc in range(4):
                        nc.tensor.matmul(pv, lhsT=xbb[:, cc, pt * 128:(pt + 1) * 128],
                                         rhs=wv_sb[:, cc, :], start=(cc == 0), stop=False)
                    nc.tensor.matmul(pv, lhsT=ones1_sb, rhs=mwvd_sb, start=False, stop=True)
                    if pt % 2 == 0:
                        nc.scalar.copy(vstage[:, pt, :], pv)
                    else:
                        nc.vector.tensor_copy(vstage[:, pt, :], pv)
                nc.sync.dma_start(out=vtw[nb], in_=vstage)

        # ---------------- Phase L: logits, exp, sums ----------------
        kc = k_sb.rearrange("c (g w) -> c g w", w=96)
        qc = q_sb.rearrange("c (g w) -> c g w", w=96)
        with ExitStack() as ph, tc.tile_pool(name="lpsum", bufs=4, space="PSUM") as pse, \
                tc.tile_pool(name="spsum", bufs=2, space="PSUM") as pss, \
                tc.tile_pool(name="sstage", bufs=2) as sst:
            for hg in range(24):
                pe4 = pse.tile([96, 384], f32, tag="pe")
                for hi in range(4):
                    h = hg * 4 + hi
                    sl = slice(hi * 96, (hi + 1) * 96)
                    nc.tensor.matmul(pe4[:, sl], lhsT=k_sb[:, h * 96:(h + 1) * 96],
                                     rhs=q_sb[:, h * 96:(h + 1) * 96],
                                     start=True, stop=True)
                nc.scalar.activation(pr_sb[:, hg * 384:(hg + 1) * 384], pe4, AF.Exp)
            for wg in range(24):
                pe4 = pse.tile([96, 384], f32, tag="pe")
                for wi in range(4):
                    w = wg * 4 + wi
                    sl = slice(wi * 96, (wi + 1) * 96)
                    nc.tensor.matmul(pe4[:, sl], lhsT=kc[:, :, w], rhs=qc[:, :, w],
                                     start=True, stop=False)
                    nc.tensor.matmul(pe4[:, sl], lhsT=ib_sb, rhs=negib_sb,
                                     start=False, stop=True)
                nc.scalar.activation(pc_sb[:, wg * 384:(wg + 1) * 384], pe4, AF.Exp)
            for j in range(NB):
                s, e = j * 512, (j + 1) * 512
                p1 = pss.tile([1, 512], f32, tag="p1")
                nc.tensor.matmul(p1, lhsT=ones96_sb, rhs=pc_sb[:, s:e], start=True, stop=True)
                t1 = sst.tile([1, 512], f32, tag="t1")
                nc.vector.tensor_copy(t1, p1)
                nc.sync.dma_start(out=sc_d[:, s:e], in_=t1)
                p2 = pss.tile([1, 512], f32, tag="p2")
                nc.tensor.matmul(p2, lhsT=ones96_sb, rhs=pr_sb[:, s:e], start=True, stop=True)
                t2 = sst.tile([1, 512], f32, tag="t2")
                nc.scalar.copy(t2, p2)
                nc.sync.dma_start(out=sr_d[:, s:e], in_=t2)

        # ---------------- Phase D: denominators -> Rg, RgT ----------------
        with ExitStack() as ph, tc.tile_pool(name="dsmall", bufs=1) as dsm, \
                tc.tile_pool(name="dpsum", bufs=1, space="PSUM") as dps:
            sct = dsm.tile([96, 96], f32)  # [w, h]
            nc.sync.dma_start(out=sct, in_=sc_d.rearrange("one (w h) -> (one w) h", h=96))
            srt = dsm.tile([96, 96], f32)  # [h, w]
            nc.sync.dma_start(out=srt, in_=sr_d.rearrange("one (h w) -> (one h) w", w=96))
            ptr = dps.tile([96, 96], f32)
            nc.tensor.transpose(ptr, sct, ib_sb)  # -> [h, w]
            d_sb = dsm.tile([96, 96], f32)
            nc.vector.tensor_add(d_sb, ptr, srt)
            r_sb = dsm.tile([96, 96], f32)
            nc.vector.reciprocal(r_sb, d_sb)
            nc.scalar.activation(rg_sb, r_sb, AF.Copy, scale=float(gamma_f))
            ptr2 = dps.tile([96, 96], f32)
            nc.tensor.transpose(ptr2, rg_sb, ib_sb)
            nc.vector.tensor_copy(rgt_sb, ptr2)

        # ------- Phases C+R interleaved: column + row attention -------
        vtc = vt_d.rearrange("(g wg wi) c -> wg g wi c", wg=24, wi=4)
        ucw = uc_d.rearrange("(h wg wi) c -> wg h wi c", wg=24, wi=4)
        vtr = vt_d.rearrange("(hg hi v) c -> hg v hi c", hg=24, hi=4)
        urw = ur_d.rearrange("(hg hi w) c -> hg w hi c", hg=24, hi=4)
        with ExitStack() as ph, tc.tile_pool(name="crstage", bufs=4) as cst, \
                tc.tile_pool(name="cpsum", bufs=3, space="PSUM") as psu, \
                tc.tile_pool(name="rpsum", bufs=3, space="PSUM") as psr:
            for grp in range(24):
                wg = grp
                vc = cst.tile([96, 4, C], bf16, tag="vc")
                nc.sync.dma_start(out=vc, in_=vtc[wg])
                uc = cst.tile([96, 4, C], bf16, tag="uc")
                for wi in range(4):
                    w = wg * 4 + wi
                    pu = psu.tile([96, C], f32, tag="pu")
                    nc.tensor.matmul(pu, lhsT=pc_sb[:, w * 96:(w + 1) * 96],
                                     rhs=vc[:, wi, :], start=True, stop=True)
                    if w % 2 == 0:
                        nc.scalar.activation(uc[:, wi, :], pu, AF.Copy,
                                             scale=rg_sb[:, w:w + 1])
                    else:
                        nc.vector.tensor_scalar_mul(uc[:, wi, :], pu, rg_sb[:, w:w + 1])
                nc.sync.dma_start(out=ucw[wg], in_=uc)
                hg = grp
                vr = cst.tile([96, 4, C], bf16, tag="vr")
                nc.sync.dma_start(out=vr, in_=vtr[hg])
                ur = cst.tile([96, 4, C], bf16, tag="ur")
                for hi in range(4):
                    h = hg * 4 + hi
                    pu = psr.tile([96, C], f32, tag="pur")
                    nc.tensor.matmul(pu, lhsT=pr_sb[:, h * 96:(h + 1) * 96],
                                     rhs=vr[:, hi, :], start=True, stop=True)
                    if h % 2 == 0:
                        nc.scalar.activation(ur[:, hi, :], pu, AF.Copy,
                                             scale=rgt_sb[:, h:h + 1])
                    else:
                        nc.vector.tensor_scalar_mul(ur[:, hi, :], pu, rgt_sb[:, h:h + 1])
                nc.sync.dma_start(out=urw[hg], in_=ur)

        # ------- Phase F: delta = uc+ur, int6 quantization + packing -------
        # delta carries the gamma/D scaling; residual add happens on host.
        # Per (channel, 512px-block): qv = round(delta * 30.5/amax) + 32 in
        # [1,63]; groups of 4 pixels pack into 3 bytes (little-endian 6-bit).
        with ExitStack() as ph, tc.tile_pool(name="fstage", bufs=3) as fst, \
                tc.tile_pool(name="fwork", bufs=1) as fwk, \
                tc.tile_pool(name="fpk", bufs=2) as fpkp, \
                tc.tile_pool(name="fsball", bufs=1) as fsb:
            for cc in range(4):
                cs = slice(cc * 128, (cc + 1) * 128)
                sball = fsb.tile([128, HW], bf16, tag="sball")
                for hb in range(6):
                    r0 = hb * 1536
                    uct = fst.tile([128, 1536], bf16, tag="uct")
                    nc.sync.dma_start(out=uct, in_=uc_d[r0:r0 + 1536, cs], transpose=True)
                    urt = fst.tile([128, 1536], bf16, tag="urt")
                    nc.sync.dma_start(out=urt, in_=ur_d[r0:r0 + 1536, cs], transpose=True)
                    if (cc + hb) % 2 == 0:
                        nc.gpsimd.tensor_add(sball[:, r0:r0 + 1536], uct, urt)
                    else:
                        nc.vector.tensor_add(sball[:, r0:r0 + 1536], uct, urt)
                amax = fwk.tile([128, NB], f32, tag="amax")
                for j in range(NB):
                    nc.vector.tensor_reduce(amax[:, j:j + 1], sball[:, j * 512:(j + 1) * 512],
                                            axis=mybir.AxisListType.X,
                                            op=mybir.AluOpType.max,
                                            apply_absolute_value=True)
                nc.sync.dma_start(out=outs_d[cs, :], in_=amax)
                rinv = fwk.tile([128, NB], f32, tag="rinv")
                nc.vector.reciprocal(rinv, amax)
                rs = fwk.tile([128, NB], f32, tag="rs")
                nc.scalar.activation(rs, rinv, AF.Copy, scale=QMARGIN)
                qv = fwk.tile([128, HW], u8, tag="qv")
                for j in range(NB):
                    nc.vector.tensor_scalar(qv[:, j * 512:(j + 1) * 512],
                                            sball[:, j * 512:(j + 1) * 512],
                                            rs[:, j:j + 1], 32.0,
                                            op0=OP.mult, op1=OP.add)
                # pack: w24 = v0 | v1<<6 | v2<<12 | v3<<18 -> 3 bytes
                g4 = qv.rearrange("p (n four) -> p n four", four=4)
                v0, v1 = g4[:, :, 0], g4[:, :, 1]
                v2, v3 = g4[:, :, 2], g4[:, :, 3]
                pk = fpkp.tile([128, HW // 4, 3], u8, tag="pk")
                t1 = fwk.tile([128, HW // 4], u8, tag="t1")
                nc.vector.tensor_scalar(t1, v1, 3, 6, op0=OP.bitwise_and,
                                        op1=OP.logical_shift_left)
                nc.vector.tensor_tensor(pk[:, :, 0], v0, t1, op=OP.bitwise_or)
                t2 = fwk.tile([128, HW // 4], u8, tag="t2")
                nc.vector.tensor_scalar(t2, v1, 2, None, op0=OP.logical_shift_right)
                t3 = fwk.tile([128, HW // 4], u8, tag="t3")
                nc.vector.tensor_scalar(t3, v2, 15, 4, op0=OP.bitwise_and,
                                        op1=OP.logical_shift_left)
                nc.vector.tensor_tensor(pk[:, :, 1], t2, t3, op=OP.bitwise_or)
                t4 = fwk.tile([128, HW // 4], u8, tag="t4")
                nc.vector.tensor_scalar(t4, v2, 4, None, op0=OP.logical_shift_right)
                t5 = fwk.tile([128, HW // 4], u8, tag="t5")
                nc.vector.tensor_scalar(t5, v3, 2, None, op0=OP.logical_shift_left)
                nc.vector.tensor_tensor(pk[:, :, 2], t4, t5, op=OP.bitwise_or)
                nc.sync.dma_start(out=outp_d[cs, :],
                                  in_=pk.rearrange("p n three -> p (n three)"))

    nc.compile()
    return nc


NGROUPS = int(__import__("os").environ.get("KERNEL_NGROUPS", "4"))


def _make_runner(gamma_f: float, ngroups: int = NGROUPS):
    """Build the Bass module once and wrap it in cached jitted dispatchers
    (the axon run_bass_kernel_spmd path, minus the per-call retrace, minus
    the host-side zero-output upload). The 8 cores are split into `ngroups`
    independent dispatch groups so a later group's upload/exec overlaps an
    earlier group's download through the shared tunnel."""
    import jax
    import jax.numpy as jnp
    from jax.sharding import Mesh, PartitionSpec, NamedSharding
    try:
        from jax.experimental.shard_map import shard_map
    except ImportError:
        from jax.shard_map import shard_map
    from concourse import bass2jax, mybir
    from concourse.bass2jax import _bass_exec_p, install_neuronx_cc_hook

    nc = _build(gamma_f)
    install_neuronx_cc_hook()
    if nc.dbg_addr is not None and nc.dbg_callbacks:
        raise RuntimeError("dbg callbacks unsupported in cached dispatch")

    partition_name = nc.partition_id_tensor.name if nc.partition_id_tensor else None
    in_names, out_names, out_avals = [], [], []
    for alloc in nc.m.functions[0].allocations:
        if not isinstance(alloc, mybir.MemoryLocationSet):
            continue
        name = alloc.memorylocations[0].name
        if alloc.kind == "ExternalInput":
            if name != partition_name:
                in_names.append(name)
        elif alloc.kind == "ExternalOutput":
            out_names.append(name)
            out_avals.append(jax.core.ShapedArray(
                tuple(alloc.tensor_shape), mybir.dt.np(alloc.dtype)))
    n_params = len(in_names)
    n_outs = len(out_names)
    bind_in_names = tuple(in_names + out_names
                          + ([partition_name] if partition_name else []))

    def _body(*args):
        operands = list(args)
        if partition_name is not None:
            operands.append(bass2jax.partition_id_tensor())
        outs = _bass_exec_p.bind(
            *operands,
            out_avals=tuple(out_avals),
            in_names=bind_in_names,
            out_names=tuple(out_names),
            lowering_input_output_aliases=(),
            sim_require_finite=True,
            sim_require_nnan=True,
            nc=nc,
        )
        return tuple(outs)

    devices = jax.devices()[:NCORES]
    assert len(devices) == NCORES, f"need {NCORES} devices, have {len(jax.devices())}"
    assert NCORES % ngroups == 0
    gsize = NCORES // ngroups
    in_specs = (PartitionSpec("core"),) * (n_params + n_outs)
    out_specs = (PartitionSpec("core"),) * n_outs
    donate = tuple(range(n_params, n_params + n_outs))
    groups = []
    for gi in range(ngroups):
        mesh = Mesh(np.asarray(devices[gi * gsize:(gi + 1) * gsize]), ("core",))
        nshard = NamedSharding(mesh, PartitionSpec("core"))
        fn = jax.jit(
            shard_map(_body, mesh=mesh, in_specs=in_specs, out_specs=out_specs,
                      check_rep=False),
            donate_argnums=donate,
            keep_unused=True,
        )
        # Donated zero output buffers, created on-device (no host upload).
        zero_fns = [
            jax.jit(
                (lambda shape, dt: (lambda: jnp.zeros(shape, dt)))(
                    (gsize * av.shape[0],) + tuple(av.shape[1:]), av.dtype),
                out_shardings=nshard)
            for av in out_avals
        ]
        groups.append(dict(fn=fn, zero_fns=zero_fns, nshard=nshard))
    return dict(nc=nc, groups=groups, gsize=gsize,
                in_names=in_names, out_names=out_names)


_cache: dict = {}
_bufs: dict = {}

try:
    import numba

    @numba.njit(cache=True, fastmath=True, nogil=True)
    def _nb_add_amax(xsrc, dlt, xd, amax):
        # xd = xsrc + dlt[c]; amax per channel row
        for c in range(xsrc.shape[0]):
            d = dlt[c]
            row = xsrc[c]
            orow = xd[c]
            m = 1e-20
            for p in range(row.shape[0]):
                t = row[p] + d
                orow[p] = t
                a = abs(t)
                if a > m:
                    m = a
            amax[c] = m

    @numba.njit(cache=True, fastmath=True, nogil=True)
    def _nb_pack7_resid(xd, amax, xp, srow, r32):
        # int7: q = rint(x*63/amax) in [-63,63]; enc = q+64; 8 vals -> 7 bytes
        for c in range(xd.shape[0]):
            m = amax[c]
            s = 63.0 / m
            si = m / 63.0
            srow[c] = si
            row = xd[c]
            rrow = r32[c]
            prow = xp[c]
            for gset in range(row.shape[0] // 8):
                w = np.int64(0)
                base = gset * 8
                for i in range(8):
                    q = np.int64(np.rint(row[base + i] * s))
                    rrow[base + i] = row[base + i] - q * si
                    w |= (q + 64) << (7 * i)
                pb = gset * 7
                for j in range(7):
                    prow[pb + j] = np.uint8((w >> (8 * j)) & 255)

    @numba.njit(cache=True, fastmath=True, nogil=True)
    def _nb_pack_c6(cb, cp, cs2):
        # cb [128, HW] -> int6 packed rows; cs2 [64, 2]: col0 = q, col1 = k
        n = cb.shape[0] // 2
        for c in range(cb.shape[0]):
            row = cb[c]
            m = 1e-20
            for p in range(row.shape[0]):
                a = abs(row[p])
                if a > m:
                    m = a
            s = 30.5 / m
            prow = cp[c]
            for g in range(row.shape[0] // 4):
                v0 = np.int32(np.rint(row[4 * g] * s)) + 32
                v1 = np.int32(np.rint(row[4 * g + 1] * s)) + 32
                v2 = np.int32(np.rint(row[4 * g + 2] * s)) + 32
                v3 = np.int32(np.rint(row[4 * g + 3] * s)) + 32
                prow[3 * g] = np.uint8((v0 | (v1 << 6)) & 255)
                prow[3 * g + 1] = np.uint8(((v1 >> 2) | (v2 << 4)) & 255)
                prow[3 * g + 2] = np.uint8(((v2 >> 4) | (v3 << 2)) & 255)
            if c < n:
                cs2[c, 0] = m / 30.5
            else:
                cs2[c - n, 1] = m / 30.5

    @numba.njit(cache=True, fastmath=True, nogil=True)
    def _nb_unpack_dequant(acc, pk, am18):
        # acc [C, HW] += unpack6(pk [C, PACKW]) scaled by am18 [C, 18]/30.5
        for c in range(acc.shape[0]):
            prow = pk[c]
            arow = acc[c]
            for j in range(18):
                step = am18[c, j] * (1.0 / 30.5)
                g0 = j * 128  # 128 groups of 4 px per 512-block
                for gg in range(128):
                    gi = g0 + gg
                    b0 = np.int32(prow[3 * gi])
                    b1 = np.int32(prow[3 * gi + 1])
                    b2 = np.int32(prow[3 * gi + 2])
                    p0 = gi * 4
                    arow[p0] += ((b0 & 63) - 32) * step
                    arow[p0 + 1] += (((b0 >> 6) | ((b1 & 15) << 2)) - 32) * step
                    arow[p0 + 2] += (((b1 >> 4) | ((b2 & 3) << 4)) - 32) * step
                    arow[p0 + 3] += ((b2 >> 2) - 32) * step
except Exception:  # numba unavailable: numpy fallbacks below
    _nb_add_amax = None
    _nb_pack7_resid = None
    _nb_pack_c6 = None
    _nb_unpack_dequant = None


def _np_pack7_resid(xd, amax, xp, srow, r32):
    si = amax * (1.0 / 63.0)
    q = np.rint(xd * (63.0 / amax)[:, None])
    np.subtract(xd, q * si[:, None], out=r32)
    srow[:] = si
    v = (q + 64).astype(np.int64).reshape(xd.shape[0], -1, 8)
    w = np.zeros(v.shape[:2], np.int64)
    for i in range(8):
        w |= v[:, :, i] << (7 * i)
    out = np.empty(v.shape[:2] + (7,), np.uint8)
    for j in range(7):
        out[:, :, j] = (w >> (8 * j)) & 255
    xp[:] = out.reshape(xp.shape)


def _np_pack_c6(cb, cp, cs2):
    n = cb.shape[0] // 2
    amax = np.maximum(np.abs(cb).max(axis=1), 1e-20)
    v = (np.rint(cb * (30.5 / amax)[:, None]).astype(np.int32) + 32)\
        .reshape(cb.shape[0], -1, 4)
    out = np.empty(v.shape[:2] + (3,), np.uint8)
    out[:, :, 0] = (v[:, :, 0] | (v[:, :, 1] << 6)) & 255
    out[:, :, 1] = ((v[:, :, 1] >> 2) | (v[:, :, 2] << 4)) & 255
    out[:, :, 2] = ((v[:, :, 2] >> 4) | (v[:, :, 3] << 2)) & 255
    cp[:] = out.reshape(cp.shape)
    cs2[:, 0] = amax[:n] * (1.0 / 30.5)
    cs2[:, 1] = amax[n:] * (1.0 / 30.5)


def _np_unpack_dequant(acc, pk, am18):
    b = pk.reshape(acc.shape[0], -1, 3).astype(np.int32)
    v = np.empty((acc.shape[0], b.shape[1], 4), np.int32)
    v[:, :, 0] = b[:, :, 0] & 63
    v[:, :, 1] = (b[:, :, 0] >> 6) | ((b[:, :, 1] & 15) << 2)
    v[:, :, 2] = (b[:, :, 1] >> 4) | ((b[:, :, 2] & 3) << 4)
    v[:, :, 3] = b[:, :, 2] >> 2
    vals = (v - 32).reshape(acc.shape[0], 18, 512).astype(np.float32)
    vals *= (am18 * (1.0 / 30.5))[:, :, None]
    acc += vals.reshape(acc.shape)


def _prep_shared(Wq, bq, Wk, bk, Wv, bv, delta):
    Wq = np.asarray(Wq, np.float32)
    Wk = np.asarray(Wk, np.float32)
    Wv = np.asarray(Wv, np.float32)
    Wqk = np.ascontiguousarray(np.vstack([Wq, Wk]))  # (2*IC, C)
    dev = dict(
        wqkT=np.ascontiguousarray(Wqk.T).astype(np.float32).reshape(4, 128, 2 * IC),
        wvT=np.ascontiguousarray(Wv.T).astype(BF).reshape(4, 128, C),
        mwvd=(-(Wv @ delta)).astype(BF).reshape(1, C),
        ib=np.eye(96, dtype=np.float32),
        negib=np.eye(96, dtype=np.float32) * -1e30,
    )
    bqk = np.concatenate([np.asarray(bq, np.float32) - Wq @ delta,
                          np.asarray(bk, np.float32) - Wk @ delta])
    return dev, Wqk, bqk


def _getbuf(name, shape, dtype):
    b = _bufs.get(name)
    if b is None or b.shape != shape or b.dtype != dtype:
        b = _bufs[name] = np.empty(shape, dtype)
    return b


_PROF = bool(__import__("os").environ.get("KERNEL_PROF"))


def _host_prep(b, x32, delta, Wqk, bqk, xd32, xp8, cp8, xs_np, cs_np, csn_np,
               r32, c32, amax):
    if _nb_add_amax is not None:
        _nb_add_amax(x32[b], delta, xd32[b], amax)
        srow = np.empty(C, np.float32)
        _nb_pack7_resid(xd32[b], amax, xp8[b], srow, r32)
    else:
        np.add(x32[b], delta[:, None], out=xd32[b])
        np.maximum(np.abs(xd32[b]).max(axis=1), 1e-20, out=amax)
        srow = np.empty(C, np.float32)
        _np_pack7_resid(xd32[b], amax, xp8[b], srow, r32)
    xs_np[b] = srow.reshape(4, 128).T
    np.matmul(Wqk, r32, out=c32)
    c32 += bqk[:, None]
    if _nb_pack_c6 is not None:
        _nb_pack_c6(c32, cp8[b], cs_np[b])
    else:
        _np_pack_c6(c32, cp8[b], cs_np[b])
    csn_np[b, 0, :IC] = -32.0 * cs_np[b, :, 0]
    csn_np[b, 1, :IC] = -32.0 * cs_np[b, :, 1]


def _run_fast(runner, x32, delta, shared, Wqk, bqk, B):
    import jax
    import time as _t
    _t0 = _t.time()
    _mark = (lambda m: print(f"  [{_t.time()-_t0:7.3f}] {m}", flush=True)) \
        if _PROF else (lambda m: None)
    groups = runner["groups"]
    gsize = runner["gsize"]
    xd32 = np.empty((B, C, HW), np.float32)  # x + gamma*bv: residual base
    xp8 = _getbuf("xp8", (B, C, XPACKW), np.uint8)
    cp8 = _getbuf("cp8", (B, 2 * IC, PACKW), np.uint8)
    xs_np = _getbuf("xs", (B, 128, 4), np.float32)
    cs_np = _getbuf("cs", (B, IC, 2), np.float32)
    csn_np = _bufs.get("csn")
    if csn_np is None:
        csn_np = _bufs["csn"] = np.zeros((B, 2, 512), np.float32)
    r32 = _getbuf("r32", (C, HW), np.float32)
    c32 = _getbuf("c32", (2 * IC, HW), np.float32)
    amax = _getbuf("amax", (C,), np.float32)
    wc = runner.get("wcache")
    fresh_w = not (wc is not None
                   and all(np.array_equal(shared[n], wc[0][n]) for n in shared))
    if fresh_w:
        dev_w_g = []
    pi = runner["out_names"].index("outp")
    si = runner["out_names"].index("outs")

    devices = jax.devices()[:NCORES]

    disp = []
    with ThreadPoolExecutor(NCORES) as ex:
        for gi, gr in enumerate(groups):
            b0, b1 = gi * gsize, (gi + 1) * gsize
            # stream each batch's planes up as soon as they're ready
            xparts, cparts = [], []
            for b in range(b0, b1):
                _host_prep(b, x32, delta, Wqk, bqk, xd32, xp8, cp8,
                           xs_np, cs_np, csn_np, r32, c32, amax)
                xparts.append(jax.device_put(xp8[b], devices[b]))
                cparts.append(jax.device_put(cp8[b], devices[b]))
            xg = jax.make_array_from_single_device_arrays(
                (gsize * C, XPACKW), gr["nshard"], xparts)
            cg = jax.make_array_from_single_device_arrays(
                (gsize * 2 * IC, PACKW), gr["nshard"], cparts)
            _mark(f"g{gi} staged")
            sg = jax.device_put(xs_np[b0:b1].reshape(gsize * 128, 4),
                                gr["nshard"])
            csg = jax.device_put(cs_np[b0:b1].reshape(gsize * IC, 2),
                                 gr["nshard"])
            csng = jax.device_put(csn_np[b0:b1].reshape(gsize * 2, 512),
                                  gr["nshard"])
            per_call = {"xp": xg, "cp": cg, "xs": sg, "cs": csg, "csn": csng}
            if fresh_w:
                dev_w = {n: jax.device_put(np.concatenate([w] * gsize, axis=0),
                                           gr["nshard"])
                         for n, w in shared.items()}
                dev_w_g.append(dev_w)
            else:
                dev_w = wc[1][gi]
            args = [per_call.get(name) if name in per_call else dev_w[name]
                    for name in runner["in_names"]]
            zeros = gr.pop("next_zeros", None) or [zf() for zf in gr["zero_fns"]]
            disp.append(gr["fn"](*args, *zeros))
            _mark(f"g{gi} dispatched")
        if fresh_w:
            runner["wcache"] = ({n: np.copy(w) for n, w in shared.items()},
                                dev_w_g)
        for gi, gr in enumerate(groups):
            # prefetch next call's donated zero buffers; overlaps exec/fetch
            gr["next_zeros"] = [zf() for zf in gr["zero_fns"]]

        jobs = []
        for gi, out_arrs in enumerate(disp):
            jobs.extend((gi, s) for s in out_arrs[pi].addressable_shards)

        def fetch(arg):
            gi, s = arg
            pk = np.asarray(s.data)  # (C, PACKW) u8; blocks until device done
            snp = np.asarray(disp[gi][si]).reshape(gsize, C, NB)
            b = gi * gsize + (s.index[0].start or 0) // C
            am18 = snp[(s.index[0].start or 0) // C]
            if _nb_unpack_dequant is not None:
                _nb_unpack_dequant(xd32[b], pk, am18)
            else:
                _np_unpack_dequant(xd32[b], pk, am18)

        list(ex.map(fetch, jobs))
        _mark("all fetched")
    return xd32


def _run_fallback(nc, x32, delta, shared, Wqk, bqk, B):
    from concourse.bass_utils import run_bass_kernel_spmd
    xd32 = x32 + delta[None, :, None]
    in_maps = []
    for b in range(B):
        xb = xd32[b]
        amax = np.maximum(np.abs(xb).max(axis=1), 1e-20)
        xp8 = np.empty((C, XPACKW), np.uint8)
        cp8 = np.empty((2 * IC, PACKW), np.uint8)
        srow = np.empty(C, np.float32)
        cs2 = np.empty((IC, 2), np.float32)
        r32 = np.empty((C, HW), np.float32)
        _np_pack7_resid(xb, amax, xp8, srow, r32)
        c32 = Wqk @ r32 + bqk[:, None]
        _np_pack_c6(c32, cp8, cs2)
        csn = np.zeros((2, 512), np.float32)
        csn[0, :IC] = -32.0 * cs2[:, 0]
        csn[1, :IC] = -32.0 * cs2[:, 1]
        in_maps.append(dict(
            shared,
            xp=xp8,
            xs=np.ascontiguousarray(srow.reshape(4, 128).T),
            cp=cp8,
            cs=cs2,
            csn=csn,
        ))
    res = run_bass_kernel_spmd(nc, in_maps, core_ids=list(range(B)))
    for b in range(B):
        _np_unpack_dequant(xd32[b], res.results[b]["outp"],
                           res.results[b]["outs"])
    return xd32


def kernel(x, Wq, bq, Wk, bk, Wv, bv, gamma):
    x = np.asarray(x)
    B = x.shape[0]
    assert B == NCORES, f"expected B={NCORES}, got {B}"
    g = float(np.asarray(gamma).reshape(-1)[0])
    delta = (g * np.asarray(bv, np.float64)).astype(np.float32)
    x32 = np.asarray(x, np.float32).reshape(B, C, HW)
    shared, Wqk, bqk = _prep_shared(Wq, bq, Wk, bk, Wv, bv, delta)

    key = round(g, 9)
    if key not in _cache:
        _cache[key] = _make_runner(g)
    runner = _cache[key]

    globals()["_last_exec_ns"] = None
    globals()["_last_trace"] = None
    try:
        res = _run_fast(runner, x32, delta, shared, Wqk, bqk, B)
    except Exception:
        import os, sys, time, traceback
        traceback.print_exc()
        if os.environ.get("KERNEL_NO_FALLBACK"):
            raise
        # A wedged NeuronCore (NRT_EXEC_UNIT_UNRECOVERABLE) persists for the
        # life of the PJRT client: tear the backend down, rebuild the runner
        # (terminal-side reconnect resets the cores), and retry once.
        print("kernel: fast dispatch failed; resetting backend", file=sys.stderr)
        try:
            import jax._src.xla_bridge as _xb
            _xb._clear_backends()
            import jax
            jax.clear_caches()
            time.sleep(2.0)
            _cache.clear()
            _cache[key] = runner = _make_runner(g)
            res = _run_fast(runner, x32, delta, shared, Wqk, bqk, B)
        except Exception:
            traceback.print_exc()
            print("kernel: retry failed; final fallback", file=sys.stderr)
            res = _run_fallback(runner["nc"], x32, delta, shared, Wqk, bqk, B)
    return res.reshape(B, C, H, W)
